# revision 1
# baseline (speedup 1.0000x reference)
"""AGCNConv (GNN message passing) distributed Bass kernel for 8 TRN2 NeuronCores.

Reference math:
    h   = x @ W
    aew = edge_weight * sigmoid(adaptive_weight)
    deg = segment_sum(aew, row);  dis = where(deg>0, deg^-1/2, 0)
    out = segment_sum(h[col] * (dis[row]*aew*dis[col])[:,None], row)
    out = LeakyReLU(LayerNorm(out + bias))

Key identities exploited:
  * The global factor s = sigmoid(adaptive_weight) cancels in the symmetric
    normalization, so adaptive_weight is unused.
  * norm_e = dis[row]*ew*dis[col] depends only on edge data -> computed on
    the HOST in _preprocess; the device never computes degrees.

Sharding: nodes (x rows / dest rows) are sharded 8 ways (5000 -> 5120 padded
rows per core); edges are routed to their destination's core and grouped by
(source window, dest block of 128), padded to 128-edge tiles with a schedule
shared across all cores (SPMD: one graph, per-core data).

Device pipeline per core:
  1. h = x @ W (PE transpose + matmul); PSUM written straight to bf16 hp.
     After each half of the blocks: replicate hp-half x8 (DMA spread over
     4 engine HWDGE streams) and AllToAll the half-shard (emulates the
     broken AllGather).
  2. Edge pass: dma_gather h'[col] rows from HBM in chunks over 4 SWDGE
     queues with 5 gather buffers in flight (int16 indices => two
     20480-row windows = first|second halves of all shards), build
     S[e,d] = (iota==dest_off[e])*norm[e] per 128-edge tile in one DVE
     tensor_scalar op (bf16), PSUM-accumulate out_blk += S^T @ G.
     Queue-parallel gathers run at ~3.7-4.5 ns/row vs 11.5 serial.
  3. Fused epilogue per block: +w0 partial, +bias, LayerNorm (ACT
     accum_out for mean/var), gamma/beta, LeakyReLU via
     scalar_tensor_tensor max(x, 0.2x), DMA out.
"""

import sys

if "/opt/trn_rl_repo" not in sys.path:
    sys.path.insert(0, "/opt/trn_rl_repo")

import numpy as np

from concourse import bacc, tile, mybir
from concourse.bass_utils import run_bass_kernel_spmd

# ---- problem constants (hardcoded per the harness contract) ----
N = 40000
E = 640000
D = 128
C = 8              # cores
NPC = 5000         # nodes per core
NB = 40            # dest blocks of 128 per core
NPCP = NB * 128    # 5120 padded nodes per core
NFULL = C * NPCP   # 40960
HALF = NPCP // 2   # 2560: half-shard rows; windows = (first|second) halves
WROWS = C * HALF   # 20480 rows per gather window (int16 index limit)
LN_EPS = 1e-5
LEAKY_ALPHA = 0.2

# ---- tunables ----
SKIP = frozenset()   # ablation flags for perf bisection: p1, x, sb, mm, g, epi
CH = 64            # gather chunk size in 128-edge tiles (8192 rows per call)
NQ = 4             # SWDGE queues for gather descriptor parallelism
GBUFS = 5          # gather buffers in flight (>= NQ+1 for queue overlap)
GDT_BF16 = True    # gather/propagate matmul in bf16 (f32 accumulate in PSUM)

_f32 = mybir.dt.float32
_bf16 = mybir.dt.bfloat16
_i16 = mybir.dt.int16


def _preprocess(edge_index, edge_weight):
    """Route edges to destination cores; group by (window, dest block); build
    the shared padded schedule and per-core tile-layout arrays.

    The full symmetric normalization norm_e = dis[row]*ew*dis[col] is
    computed HERE on the host (it depends only on edge data), so the device
    never computes degrees: S tiles carry norm directly."""
    row = np.asarray(edge_index[0], dtype=np.int64)
    col = np.asarray(edge_index[1], dtype=np.int64)
    ew = np.asarray(edge_weight, dtype=np.float32)
    deg = np.bincount(row, weights=ew, minlength=N)
    dis = np.where(deg > 0, deg ** -0.5, 0.0)
    ew = (dis[row] * ew * dis[col]).astype(np.float32)

    core = row // NPC
    lid = row - core * NPC
    blk = lid >> 7
    off = (lid & 127).astype(np.float32)
    s = col // NPC
    r = col - s * NPC                        # 0..4999 within shard
    win = (r >= HALF).astype(np.int64)       # window A: first half of every
    crel = (s * HALF + (r - win * HALF)).astype(np.int16)  # shard; B: second

    # group key per edge: (core, window, block)
    key = (core * 2 + win) * NB + blk
    counts = np.bincount(key, minlength=C * 2 * NB).reshape(C, 2, NB)
    # shared schedule: tiles per (window, block) = max over cores, rounded up
    P = 128 * np.ceil(counts.max(axis=0) / 128.0).astype(np.int64)  # [2, NB]
    for b in range(NB):
        if P[0, b] + P[1, b] == 0:
            P[1, b] = 128  # guarantee at least one (all-dummy) tile per block

    Ppad = int(P.sum())
    T = Ppad // 128
    gstart = np.concatenate([[0], np.cumsum(P.reshape(-1))])[:-1].reshape(2, NB)

    colr_a = np.zeros((C, Ppad), np.int16)
    ew_a = np.zeros((C, Ppad), np.float32)
    dof_a = np.zeros((C, Ppad), np.float32)

    order = np.argsort(key, kind="stable")
    key_s = key[order]
    # position within each (c,w,b) group
    grp_sizes = counts.reshape(-1)
    grp_off = np.concatenate([[0], np.cumsum(grp_sizes)])[:-1]
    within = np.arange(len(key_s)) - grp_off[key_s]
    c_s = key_s // (2 * NB)
    wb_s = key_s % (2 * NB)
    dest = gstart.reshape(-1)[wb_s] + within
    colr_a[c_s, dest] = crel[order]
    ew_a[c_s, dest] = ew[order]
    dof_a[c_s, dest] = off[order]

    # tile-major layouts
    ew_t = np.ascontiguousarray(ew_a.reshape(C, T, 128).transpose(0, 2, 1))      # [C,128,T]
    dof_t = np.ascontiguousarray(dof_a.reshape(C, T, 128).transpose(0, 2, 1))    # [C,128,T]
    # dma_gather index layout: idx i of a tile at [i%16, i//16], replicated x8
    A = colr_a.reshape(C, T, 8, 16).transpose(0, 1, 3, 2)                        # [C,T,16,8]
    idx16 = np.tile(A, (1, 1, 8, 1)).transpose(0, 2, 1, 3).reshape(C, 128, 8 * T)
    idx16 = np.ascontiguousarray(idx16)

    return P, T, ew_t, dof_t, idx16


def _schedule(P):
    """Static tile schedule shared by all cores."""
    P = np.asarray(P)
    tiles = []  # (global tile idx) -> (w, b)
    for w in (0, 1):
        for b in range(NB):
            for _ in range(int(P[w, b]) // 128):
                tiles.append((w, b))
    T = len(tiles)
    t0w = [0, int(P[0].sum()) // 128]
    Tw = [t0w[1], T - t0w[1]]
    return tiles, t0w, Tw


def _build(P, T, tiles, t0w, Tw, sim_single_core=False, reps=1):
    gdt = _bf16 if GDT_BF16 else _f32
    # SBUF budget (KB/partition): resident tables grow with T (skewed graphs)
    idx_kb = (16 * T) / 1024.0 if T <= 2560 else 2.0
    fixed_kb = 52 + idx_kb + (8 * T) / 1024.0  # hp/acc/consts + tables
    ch, gbufs = CH, GBUFS
    while fixed_kb + ch * 0.25 * gbufs + 20 > 155 and (ch > 4 or gbufs > 4):
        if gbufs > 4:
            gbufs -= 1
        else:
            ch //= 2
    nc = bacc.Bacc("TRN2", target_bir_lowering=False, debug=False,
                   enable_asserts=True,
                   num_devices=1 if sim_single_core else C,
                   num_swdge_queues=NQ, dynamic_dma_scratch_size=65536)

    x_in = nc.dram_tensor("x", [NPCP, D], _f32, kind="ExternalInput").ap()
    w_in = nc.dram_tensor("w", [D, D], _f32, kind="ExternalInput").ap()
    bias_in = nc.dram_tensor("bias", [1, D], _f32, kind="ExternalInput").ap()
    gamma_in = nc.dram_tensor("gamma", [1, D], _f32, kind="ExternalInput").ap()
    beta_in = nc.dram_tensor("beta", [1, D], _f32, kind="ExternalInput").ap()
    ewt_in = nc.dram_tensor("ewt", [128, T], _f32, kind="ExternalInput").ap()
    doft_in = nc.dram_tensor("doft", [128, T], _f32, kind="ExternalInput").ap()
    idx_in = nc.dram_tensor("idx16", [128, 8 * T], _i16, kind="ExternalInput").ap()
    out_d = nc.dram_tensor("out", [NPCP, D], _f32, kind="ExternalOutput").ap()

    eq = mybir.AluOpType.is_equal
    mul = mybir.AluOpType.mult
    add = mybir.AluOpType.add
    AF = mybir.ActivationFunctionType

    # per-block window runs: (first_tile, last_tile) or None
    runs = [[None, None] for _ in range(NB)]
    for t, (w, b) in enumerate(tiles):
        if runs[b][w] is None:
            runs[b][w] = [t, t]
        else:
            runs[b][w][1] = t

    with tile.TileContext(nc) as tc:
        with (
            tc.tile_pool(name="const", bufs=1) as cp,
            tc.tile_pool(name="resident", bufs=1) as rp,
            tc.tile_pool(name="dram", bufs=1, space="DRAM") as dp,
        ):
            iota = cp.tile([128, 128], _f32)
            nc.gpsimd.iota(iota[:], pattern=[[1, 128]], base=0,
                           channel_multiplier=0,
                           allow_small_or_imprecise_dtypes=True)
            pidx = cp.tile([128, 1], _f32)
            nc.gpsimd.iota(pidx[:], pattern=[[0, 1]], base=0,
                           channel_multiplier=1,
                           allow_small_or_imprecise_dtypes=True)
            ident = cp.tile([128, 128], _f32)
            nc.vector.tensor_scalar(ident[:], iota[:], pidx[:], None, op0=eq)
            iota_bf = cp.tile([128, 128], _bf16)
            nc.vector.tensor_copy(iota_bf[:], iota[:])
            ones_row = cp.tile([1, 128], _f32)
            nc.vector.memset(ones_row[:], 1.0)
            eps_col = cp.tile([128, 1], _f32)
            nc.vector.memset(eps_col[:], float(LN_EPS))
            w_sb = cp.tile([128, 128], _f32)
            nc.sync.dma_start(w_sb[:], w_in)

            # broadcast bias/gamma/beta rows to all 128 partitions via matmul
            bias_bc = cp.tile([128, 128], _f32)
            gamma_bc = cp.tile([128, 128], _f32)
            beta_bc = cp.tile([128, 128], _f32)
            with tc.tile_pool(name="bc", bufs=1) as bcp, \
                 tc.tile_pool(name="bcps", bufs=1, space="PSUM") as bcps:
                for src_ap, dst in ((bias_in, bias_bc), (gamma_in, gamma_bc),
                                    (beta_in, beta_bc)):
                    r = bcp.tile([1, 128], _f32, tag="bcrow")
                    nc.sync.dma_start(r[:], src_ap)
                    ps = bcps.tile([128, 128], _f32, tag="bcps")
                    nc.tensor.matmul(ps[:], lhsT=ones_row[:], rhs=r[:],
                                     start=True, stop=True)
                    nc.scalar.copy(dst[:], ps[:])

            ewt = rp.tile([128, T], _f32)
            nc.sync.dma_start(ewt[:], ewt_in)
            doft = rp.tile([128, T], _f32)
            nc.sync.dma_start(doft[:], doft_in)

            idx_resident = T <= 2560
            if idx_resident:
                idx_sb = rp.tile([128, 8 * T], _i16)
                nc.sync.dma_start(idx_sb[:], idx_in)

            hp_sb = rp.tile([128, NB, 128], gdt)    # node features, gather dtype
            acc_sb = rp.tile([128, NB, 128], _f32)  # window-0 partial sums

            a2a_in1 = dp.tile([C * HALF, D], gdt)
            a2a_in2 = dp.tile([C * HALF, D], gdt)
            hfullA = dp.tile([WROWS, D], gdt)
            hfullB = dp.tile([WROWS, D], gdt)

            # engines with an independent HWDGE stream for plain DMAs
            dmaeng = [nc.sync, nc.scalar, nc.gpsimd]

            def _phases():
                def exchange(h):
                    # replicate hp half x8 + A2A for node-half h
                    if "x" in SKIP:
                        return
                    lo, hi = h * (NB // 2), (h + 1) * (NB // 2)
                    a2a = a2a_in1 if h == 0 else a2a_in2
                    hf = hfullA if h == 0 else hfullB
                    for rr in range(C):
                        dst = a2a[rr * HALF:(rr + 1) * HALF, :].rearrange(
                            "(t p) d -> p t d", p=128)
                        dmaeng[rr % len(dmaeng)].dma_start(
                            dst, hp_sb[:, lo:hi, :])
                    if sim_single_core:
                        q = C * HALF // 4
                        for k in range(4):
                            dmaeng[k % len(dmaeng)].dma_start(
                                hf[k * q:(k + 1) * q, :],
                                a2a[k * q:(k + 1) * q, :])
                    else:
                        nc.gpsimd.collective_compute(
                            "AllToAll", mybir.AluOpType.bypass,
                            replica_groups=[list(range(C))],
                            ins=[a2a.opt()], outs=[hf.opt()])

                # ---------------- Phase 1: hp = bf16(x @ W) ----------------
                with tc.tile_pool(name="p1", bufs=4) as p1, \
                     tc.tile_pool(name="p1ps", bufs=4, space="PSUM") as p1ps:
                    if "p1" in SKIP:
                        nc.vector.memset(hp_sb[:, 0, :1], 0.0)
                        exchange(0)
                        exchange(1)
                    for nb in range(NB):
                        if "p1" in SKIP:
                            break
                        xt = p1.tile([128, 128], _f32, tag="xt")
                        dmaeng[nb % 2].dma_start(
                            xt[:], x_in[nb * 128:(nb + 1) * 128, :])
                        xTp = p1ps.tile([128, 128], _f32, tag="xT")
                        nc.tensor.transpose(xTp[:], xt[:], ident[:])
                        xTs = p1.tile([128, 128], _f32, tag="xTs")
                        nc.scalar.copy(xTs[:], xTp[:])
                        hp = p1ps.tile([128, 128], _f32, tag="hps")
                        nc.tensor.matmul(hp[:], lhsT=xTs[:], rhs=w_sb[:],
                                         start=True, stop=True)
                        nc.scalar.copy(hp_sb[:, nb, :], hp[:])
                        if nb == NB // 2 - 1:
                            exchange(0)
                        elif nb == NB - 1:
                            exchange(1)

                # ---------------- Edge pass: gather + scatter matmuls --------
                with tc.tile_pool(name="pb", bufs=8) as pb, \
                     tc.tile_pool(name="gb", bufs=gbufs) as gbp, \
                     tc.tile_pool(name="pbps", bufs=4, space="PSUM") as pbps, \
                     tc.tile_pool(name="ep", bufs=2) as ep:

                    def epilogue(b, ps):
                        if "epi" in SKIP:
                            return
                        has0 = runs[b][0] is not None
                        if has0 and ps is not None:
                            t4 = ep.tile([128, 128], _f32, tag="e_t4")
                            nc.vector.tensor_tensor(t4[:], ps[:], acc_sb[:, b, :], op=add)
                            nc.vector.tensor_tensor(t4[:], t4[:], bias_bc[:], op=add)
                        elif ps is not None:
                            t4 = ep.tile([128, 128], _f32, tag="e_t4")
                            nc.vector.tensor_tensor(t4[:], ps[:], bias_bc[:], op=add)
                        else:
                            t4 = ep.tile([128, 128], _f32, tag="e_t4")
                            nc.vector.tensor_tensor(t4[:], acc_sb[:, b, :],
                                                    bias_bc[:], op=add)
                        nsum = ep.tile([128, 1], _f32, tag="e_ns")
                        nc.vector.tensor_reduce(nsum[:], t4[:],
                                                axis=mybir.AxisListType.X,
                                                op=add, negate=True)
                        nmean = ep.tile([128, 1], _f32, tag="e_nm")
                        nc.scalar.mul(nmean[:], nsum[:], 1.0 / 128.0)
                        t5 = ep.tile([128, 128], _f32, tag="e_t5")
                        nc.scalar.activation(t5[:], t4[:], AF.Identity,
                                             bias=nmean[:], scale=1.0)
                        sq = ep.tile([128, 128], _f32, tag="e_sq")
                        vsum = ep.tile([128, 1], _f32, tag="e_vs")
                        nc.scalar.activation(sq[:], t5[:], AF.Square,
                                             accum_out=vsum[:])
                        sd = ep.tile([128, 1], _f32, tag="e_sd")
                        nc.scalar.activation(sd[:], vsum[:], AF.Sqrt,
                                             scale=1.0 / 128.0, bias=eps_col[:])
                        rstd = ep.tile([128, 1], _f32, tag="e_rs")
                        nc.vector.reciprocal(rstd[:], sd[:])
                        t6 = ep.tile([128, 128], _f32, tag="e_t6")
                        nc.vector.scalar_tensor_tensor(t6[:], t5[:], rstd[:],
                                                       gamma_bc[:], op0=mul, op1=mul)
                        nc.vector.tensor_tensor(t6[:], t6[:], beta_bc[:], op=add)
                        osb = ep.tile([128, 128], _f32, tag="e_o")
                        nc.vector.scalar_tensor_tensor(osb[:], t6[:],
                                                       float(LEAKY_ALPHA), t6[:],
                                                       op0=mul,
                                                       op1=mybir.AluOpType.max)
                        nc.sync.dma_start(out_d[b * 128:(b + 1) * 128, :], osb[:])

                    gcall = [0]
                    for w in (0, 1):
                        win_ap = (hfullA if w == 0 else hfullB)[:]
                        nt_left = Tw[w]
                        t0 = t0w[w]
                        cur_ps = None
                        cur_b = -1
                        while nt_left > 0:
                            if nt_left > ch:
                                nt = ch
                            elif w == 1 and nt_left > 24:
                                nt = nt_left - 16
                            else:
                                nt = nt_left
                            gbuf = gbp.tile([128, ch, 128], gdt, tag="g")
                            if idx_resident:
                                idx_ap = idx_sb[:, 8 * t0:8 * (t0 + nt)]
                            else:
                                idx_ch = gbp.tile([128, 8 * ch], _i16, tag="ix")
                                nc.sync.dma_start(idx_ch[:, :8 * nt],
                                                  idx_in[:, 8 * t0:8 * (t0 + nt)])
                                idx_ap = idx_ch[:, :8 * nt]
                            if "g" in SKIP:
                                nc.vector.memset(gbuf[:, 0, :1], 0.0)
                            if "g" not in SKIP:
                                nc.gpsimd.dma_gather(
                                    out_ap=gbuf[:, :nt, :], in_ap=win_ap,
                                    idxs_ap=idx_ap,
                                    num_idxs=128 * nt, num_idxs_reg=128 * nt,
                                    elem_size=128, single_packet=False,
                                    queue_num=gcall[0] % NQ)
                                gcall[0] += 1
                            for s_i in range(nt):
                                t = t0 + s_i
                                b = tiles[t][1]
                                first = runs[b][w][0] == t
                                last = runs[b][w][1] == t
                                if first:
                                    cur_ps = pbps.tile([128, 128], _f32, tag="blk")
                                    cur_b = b
                                    if "mm" in SKIP:
                                        nc.vector.memset(cur_ps[:, :1], 0.0)
                                S = pb.tile([128, 128], gdt, tag="S")
                                if "sb" not in SKIP:
                                    nc.vector.tensor_scalar(S[:], iota_bf[:],
                                                            doft[:, t:t + 1],
                                                            ewt[:, t:t + 1],
                                                            op0=eq, op1=mul)
                                if "mm" not in SKIP:
                                    nc.tensor.matmul(cur_ps[:], lhsT=S[:],
                                                     rhs=gbuf[:, s_i, :],
                                                     start=first, stop=last)
                                if last:
                                    if w == 0:
                                        nc.scalar.copy(acc_sb[:, b, :], cur_ps[:])
                                    else:
                                        epilogue(b, cur_ps)
                                    cur_ps = None
                            t0 += nt
                            nt_left -= nt
                    # blocks with no window-1 tiles: epilogue from acc only
                    for b in range(NB):
                        if runs[b][1] is None:
                            epilogue(b, None)

            if reps == 1:
                _phases()
            else:
                with tc.For_i(0, reps, 1):
                    _phases()

    nc.compile()
    return nc


_CACHE = {}


def _get_compiled(edge_index, edge_weight):
    P, T, ew_t, dof_t, idx16 = _preprocess(edge_index, edge_weight)
    key = P.tobytes()
    if key not in _CACHE:
        tiles, t0w, Tw = _schedule(P)
        _CACHE[key] = _build(P, T, tiles, t0w, Tw)
    return _CACHE[key], ew_t, dof_t, idx16


def kernel(x, edge_index, edge_weight, weight, adaptive_weight, bias,
           ln_gamma, ln_beta):
    x = np.asarray(x, dtype=np.float32)
    weight = np.asarray(weight, dtype=np.float32)
    bias = np.asarray(bias, dtype=np.float32).reshape(1, D)
    gamma = np.asarray(ln_gamma, dtype=np.float32).reshape(1, D)
    beta = np.asarray(ln_beta, dtype=np.float32).reshape(1, D)

    nc, ew_t, dof_t, idx16 = _get_compiled(edge_index, edge_weight)

    in_maps = []
    for c in range(C):
        xp = np.zeros((NPCP, D), np.float32)
        xp[:NPC] = x[c * NPC:(c + 1) * NPC]
        in_maps.append({
            "x": xp, "w": weight, "bias": bias, "gamma": gamma, "beta": beta,
            "ewt": np.ascontiguousarray(ew_t[c]),
            "doft": np.ascontiguousarray(dof_t[c]),
            "idx16": np.ascontiguousarray(idx16[c]),
        })

    res = run_bass_kernel_spmd(nc, in_maps, core_ids=list(range(C)))
    out = np.empty((N, D), np.float32)
    for c in range(C):
        out[c * NPC:(c + 1) * NPC] = res.results[c]["out"][:NPC]
    return out



# revision 3
# speedup vs baseline: 1.0186x; 1.0186x over previous
"""AGCNConv (GNN message passing) distributed Bass kernel for 8 TRN2 NeuronCores.

Reference math:
    h   = x @ W
    aew = edge_weight * sigmoid(adaptive_weight)
    deg = segment_sum(aew, row);  dis = where(deg>0, deg^-1/2, 0)
    out = segment_sum(h[col] * (dis[row]*aew*dis[col])[:,None], row)
    out = LeakyReLU(LayerNorm(out + bias))

Key identities exploited:
  * The global factor s = sigmoid(adaptive_weight) cancels in the symmetric
    normalization, so adaptive_weight is unused.
  * norm_e = dis[row]*ew*dis[col] depends only on edge data -> computed on
    the HOST in _preprocess; the device never computes degrees.

v2 design (no collectives): every core receives the FULL transposed node
matrix xT (bf16, replicated input) and redundantly computes h = x @ W for
all 40960 padded source rows, writing bf16 h to its local DRAM in two
20480-row windows (the int16 gather-index limit). Dest nodes are sharded
8 ways (5120 padded rows per core); edges are routed to their dest core
and grouped by (source window, dest block of 128), sorted by source row
within each group (gather locality), padded to 128-edge tiles with a
schedule shared across all cores (SPMD: one program, per-core tables).

Device pipeline per core:
  1. h window w: xT chunks DMA'd in, 4 matmuls per [128,512] PSUM bank
     (lhsT = xT chunk, rhs = W, both bf16), one ACT copy -> bf16 staging,
     one DMA per 512 rows into h[w] DRAM.
  2. Edge pass window w: dma_gather h[col] rows (4 SWDGE queues rotating,
     CH-tile chunks, GBUFS buffers in flight; measured ~2.9 ns/row), build
     S[e,d] = (iota==dest_off[e])*norm[e] per 128-edge tile in one DVE
     tensor_scalar op (bf16), PSUM-accumulate out_blk += S^T @ G per dest
     block. Window-0 partials parked in SBUF (acc_sb); window-1 completion
     triggers the fused epilogue.
  3. Epilogue per block: +acc, +bias, LayerNorm (ACT accum_out for
     mean/var), gamma/beta, LeakyReLU via scalar_tensor_tensor
     max(x, 0.2x), DMA out.
"""

import sys

if "/opt/trn_rl_repo" not in sys.path:
    sys.path.insert(0, "/opt/trn_rl_repo")

import numpy as np
import ml_dtypes

from concourse import bacc, tile, mybir
from concourse.bass_utils import run_bass_kernel_spmd

# ---- problem constants (hardcoded per the harness contract) ----
N = 40000
E = 640000
D = 128
C = 8                # cores
NPC = 5000           # dest nodes per core
NB = 40              # dest blocks of 128 per core
NPCP = NB * 128      # 5120 padded dest rows per core
SRC_PAD = C * NPCP   # 40960 padded source rows
WIN = SRC_PAD // 2   # 20480 rows per gather window (int16 index limit)
LN_EPS = 1e-5
LEAKY_ALPHA = 0.2

# ---- tunables ----
SKIP = frozenset()   # ablation flags: p1, g, sb, mm, epi
CH = 64              # gather chunk size in 128-edge tiles
NQ = 4               # SWDGE queues (ucode max 4)
GBUFS = 5            # gather buffers in flight
XCH = 16             # node tiles per xT chunk DMA (2048 nodes)
PST = 4              # node tiles per P1 PSUM group ([128,512] bank)

_f32 = mybir.dt.float32
_bf16 = mybir.dt.bfloat16
_i16 = mybir.dt.int16


def _preprocess(edge_index, edge_weight):
    """Host: symmetric normalization, edge routing/grouping, shared padded
    schedule, per-core tile-layout tables (ewt/doft/idx16)."""
    row = np.asarray(edge_index[0], dtype=np.int64)
    col = np.asarray(edge_index[1], dtype=np.int64)
    ew = np.asarray(edge_weight, dtype=np.float32)
    deg = np.bincount(row, weights=ew, minlength=N)
    dis = np.where(deg > 0, deg ** -0.5, 0.0)
    ew = (dis[row] * ew * dis[col]).astype(np.float32)

    core = row // NPC
    lid = row - core * NPC
    blk = lid >> 7
    off = (lid & 127).astype(np.float32)
    win = col // WIN                          # source window 0|1
    crel = (col - win * WIN).astype(np.int16)  # 0..20479

    # group key per edge: (core, window, block)
    key = (core * 2 + win) * NB + blk
    counts = np.bincount(key, minlength=C * 2 * NB).reshape(C, 2, NB)
    # shared schedule: tiles per (window, block) = max over cores, rounded up
    P = 128 * np.ceil(counts.max(axis=0) / 128.0).astype(np.int64)  # [2, NB]
    for b in range(NB):
        if P[0, b] + P[1, b] == 0:
            P[1, b] = 128  # at least one (all-dummy) tile per block

    Ppad = int(P.sum())
    T = Ppad // 128
    gstart = np.concatenate([[0], np.cumsum(P.reshape(-1))])[:-1].reshape(2, NB)

    colr_a = np.zeros((C, Ppad), np.int16)
    ew_a = np.zeros((C, Ppad), np.float32)
    dof_a = np.zeros((C, Ppad), np.float32)

    # sort by (group, source row): order within a group is free, ascending
    # source addresses give the gather slightly better DRAM locality
    order = np.lexsort((crel, key))
    key_s = key[order]
    grp_sizes = counts.reshape(-1)
    grp_off = np.concatenate([[0], np.cumsum(grp_sizes)])[:-1]
    within = np.arange(len(key_s)) - grp_off[key_s]
    c_s = key_s // (2 * NB)
    wb_s = key_s % (2 * NB)
    dest = gstart.reshape(-1)[wb_s] + within
    colr_a[c_s, dest] = crel[order]
    ew_a[c_s, dest] = ew[order]
    dof_a[c_s, dest] = off[order]

    # tile-major layouts
    ew_t = np.ascontiguousarray(ew_a.reshape(C, T, 128).transpose(0, 2, 1))    # [C,128,T]
    dof_t = np.ascontiguousarray(dof_a.reshape(C, T, 128).transpose(0, 2, 1))  # [C,128,T]
    # dma_gather index layout: idx i of a tile at [i%16, i//16], replicated x8
    A = colr_a.reshape(C, T, 8, 16).transpose(0, 1, 3, 2)                      # [C,T,16,8]
    idx16 = np.tile(A, (1, 1, 8, 1)).transpose(0, 2, 1, 3).reshape(C, 128, 8 * T)
    return P, T, ew_t, dof_t, np.ascontiguousarray(idx16)


def _schedule(P):
    """Static tile schedule shared by all cores."""
    P = np.asarray(P)
    tiles = []  # global tile idx -> (w, b)
    for w in (0, 1):
        for b in range(NB):
            for _ in range(int(P[w, b]) // 128):
                tiles.append((w, b))
    T = len(tiles)
    t0w = [0, int(P[0].sum()) // 128]
    Tw = [t0w[1], T - t0w[1]]
    return tiles, t0w, Tw


def _build(P, T, tiles, t0w, Tw, sim_single_core=False, reps=1):
    del sim_single_core  # v2 has no collectives; kept for test.py compat
    nc = bacc.Bacc("TRN2", target_bir_lowering=False, debug=False,
                   enable_asserts=True, num_devices=1,
                   num_swdge_queues=NQ, dynamic_dma_scratch_size=65536)

    xt_in = nc.dram_tensor("xt", [128, SRC_PAD], _bf16, kind="ExternalInput").ap()
    w_in = nc.dram_tensor("w", [D, D], _bf16, kind="ExternalInput").ap()
    bias_in = nc.dram_tensor("bias", [1, D], _f32, kind="ExternalInput").ap()
    gamma_in = nc.dram_tensor("gamma", [1, D], _f32, kind="ExternalInput").ap()
    beta_in = nc.dram_tensor("beta", [1, D], _f32, kind="ExternalInput").ap()
    ewt_in = nc.dram_tensor("ewt", [128, T], _f32, kind="ExternalInput").ap()
    doft_in = nc.dram_tensor("doft", [128, T], _f32, kind="ExternalInput").ap()
    idx_in = nc.dram_tensor("idx16", [128, 8 * T], _i16, kind="ExternalInput").ap()
    out_d = nc.dram_tensor("out", [NPCP, D], _f32, kind="ExternalOutput").ap()

    eq = mybir.AluOpType.is_equal
    mul = mybir.AluOpType.mult
    add = mybir.AluOpType.add
    AF = mybir.ActivationFunctionType

    # per-block window runs: (first_tile, last_tile) or None
    runs = [[None, None] for _ in range(NB)]
    for t, (w, b) in enumerate(tiles):
        if runs[b][w] is None:
            runs[b][w] = [t, t]
        else:
            runs[b][w][1] = t

    with tile.TileContext(nc) as tc:
        with (
            tc.tile_pool(name="const", bufs=1) as cp,
            tc.tile_pool(name="resident", bufs=1) as rp,
            tc.tile_pool(name="dram", bufs=1, space="DRAM") as dp,
        ):
            iota = cp.tile([128, 128], _f32)
            nc.gpsimd.iota(iota[:], pattern=[[1, 128]], base=0,
                           channel_multiplier=0,
                           allow_small_or_imprecise_dtypes=True)
            iota_bf = cp.tile([128, 128], _bf16)
            nc.vector.tensor_copy(iota_bf[:], iota[:])
            ones_row = cp.tile([1, 128], _f32)
            nc.vector.memset(ones_row[:], 1.0)
            eps_col = cp.tile([128, 1], _f32)
            nc.vector.memset(eps_col[:], float(LN_EPS))
            w_sb = cp.tile([128, 128], _bf16)
            nc.sync.dma_start(w_sb[:], w_in)

            # broadcast bias/gamma/beta rows to all 128 partitions via matmul
            bias_bc = cp.tile([128, 128], _f32)
            gamma_bc = cp.tile([128, 128], _f32)
            beta_bc = cp.tile([128, 128], _f32)
            with tc.tile_pool(name="bc", bufs=1) as bcp, \
                 tc.tile_pool(name="bcps", bufs=1, space="PSUM") as bcps:
                for src_ap, dst in ((bias_in, bias_bc), (gamma_in, gamma_bc),
                                    (beta_in, beta_bc)):
                    r = bcp.tile([1, 128], _f32, tag="bcrow")
                    nc.sync.dma_start(r[:], src_ap)
                    ps = bcps.tile([128, 128], _f32, tag="bcps")
                    nc.tensor.matmul(ps[:], lhsT=ones_row[:], rhs=r[:],
                                     start=True, stop=True)
                    nc.scalar.copy(dst[:], ps[:])

            ewt = rp.tile([128, T], _f32)
            nc.sync.dma_start(ewt[:], ewt_in)
            doft = rp.tile([128, T], _f32)
            nc.sync.dma_start(doft[:], doft_in)
            idx_sb = rp.tile([128, 8 * T], _i16)
            nc.sync.dma_start(idx_sb[:], idx_in)

            acc_sb = rp.tile([128, NB, 128], _f32)  # window-0 partial sums

            h_a = dp.tile([WIN, D], _bf16)
            h_b = dp.tile([WIN, D], _bf16)
            h_d = [h_a, h_b]

            dmaeng = [nc.sync, nc.scalar]  # HWDGE streams for plain DMAs

            def _phases():
                # ---------------- Phase 1: h[w] = bf16(x @ W) ----------------
                def p1_window(w):
                    if "p1" in SKIP:
                        return
                    nchunks = WIN // (XCH * 128)
                    for ci in range(nchunks):
                        base = w * WIN + ci * XCH * 128
                        xc = p1x.tile([128, XCH * 128], _bf16, tag="xc")
                        dmaeng[ci % 2].dma_start(
                            xc[:], xt_in[:, base:base + XCH * 128])
                        for g in range(XCH // PST):
                            ps = p1ps.tile([128, PST * 128], _f32, tag="hps")
                            for j in range(PST):
                                k = (g * PST + j) * 128
                                nc.tensor.matmul(
                                    ps[:, j * 128:(j + 1) * 128],
                                    lhsT=xc[:, k:k + 128], rhs=w_sb[:],
                                    start=True, stop=True)
                            hst = p1h.tile([128, PST, 128], _bf16, tag="hst")
                            nc.scalar.copy(hst[:], ps[:])
                            r0 = ci * XCH * 128 + g * PST * 128
                            dst = h_d[w][r0:r0 + PST * 128, :].rearrange(
                                "(t p) d -> p t d", p=128)
                            dmaeng[(ci + g) % 2].dma_start(dst, hst[:])

                # ---------------- Edge pass: gather + scatter matmuls --------
                def epilogue(b, ps):
                    if "epi" in SKIP:
                        return
                    has0 = runs[b][0] is not None
                    t4 = ep.tile([128, 128], _f32, tag="e_t4")
                    if has0 and ps is not None:
                        nc.vector.tensor_tensor(t4[:], ps[:], acc_sb[:, b, :],
                                                op=add)
                        nc.vector.tensor_tensor(t4[:], t4[:], bias_bc[:], op=add)
                    elif ps is not None:
                        nc.vector.tensor_tensor(t4[:], ps[:], bias_bc[:], op=add)
                    else:
                        nc.vector.tensor_tensor(t4[:], acc_sb[:, b, :],
                                                bias_bc[:], op=add)
                    nsum = ep.tile([128, 1], _f32, tag="e_ns")
                    nc.vector.tensor_reduce(nsum[:], t4[:],
                                            axis=mybir.AxisListType.X,
                                            op=add, negate=True)
                    nmean = ep.tile([128, 1], _f32, tag="e_nm")
                    nc.scalar.mul(nmean[:], nsum[:], 1.0 / 128.0)
                    t5 = ep.tile([128, 128], _f32, tag="e_t5")
                    nc.scalar.activation(t5[:], t4[:], AF.Identity,
                                         bias=nmean[:], scale=1.0)
                    sq = ep.tile([128, 128], _f32, tag="e_sq")
                    vsum = ep.tile([128, 1], _f32, tag="e_vs")
                    nc.scalar.activation(sq[:], t5[:], AF.Square,
                                         accum_out=vsum[:])
                    sd = ep.tile([128, 1], _f32, tag="e_sd")
                    nc.scalar.activation(sd[:], vsum[:], AF.Sqrt,
                                         scale=1.0 / 128.0, bias=eps_col[:])
                    rstd = ep.tile([128, 1], _f32, tag="e_rs")
                    nc.vector.reciprocal(rstd[:], sd[:])
                    t6 = ep.tile([128, 128], _f32, tag="e_t6")
                    nc.vector.scalar_tensor_tensor(t6[:], t5[:], rstd[:],
                                                   gamma_bc[:], op0=mul, op1=mul)
                    nc.vector.tensor_tensor(t6[:], t6[:], beta_bc[:], op=add)
                    osb = ep.tile([128, 128], _f32, tag="e_o")
                    nc.vector.scalar_tensor_tensor(osb[:], t6[:],
                                                   float(LEAKY_ALPHA), t6[:],
                                                   op0=mul,
                                                   op1=mybir.AluOpType.max)
                    nc.sync.dma_start(out_d[b * 128:(b + 1) * 128, :], osb[:])

                gcall = [0]

                def edge_window(w):
                    win_ap = h_d[w][:]
                    nt_left = Tw[w]
                    t0 = t0w[w]
                    cur_ps = None
                    while nt_left > 0:
                        if nt_left > CH:
                            nt = CH
                        elif w == 1 and nt_left > 24:
                            nt = nt_left - 16
                        else:
                            nt = nt_left
                        gbuf = gbp.tile([128, CH, 128], _bf16, tag="g")
                        if "g" in SKIP:
                            nc.vector.memset(gbuf[:], 0.0)
                        else:
                            nc.gpsimd.dma_gather(
                                out_ap=gbuf[:, :nt, :], in_ap=win_ap,
                                idxs_ap=idx_sb[:, 8 * t0:8 * (t0 + nt)],
                                num_idxs=128 * nt, num_idxs_reg=128 * nt,
                                elem_size=128, single_packet=False,
                                queue_num=gcall[0] % NQ)
                            gcall[0] += 1
                        for s_i in range(nt):
                            t = t0 + s_i
                            b = tiles[t][1]
                            first = runs[b][w][0] == t
                            last = runs[b][w][1] == t
                            if first:
                                cur_ps = pbps.tile([128, 128], _f32, tag="blk")
                                if "mm" in SKIP:
                                    nc.vector.memset(cur_ps[:], 0.0)
                            S = pb.tile([128, 128], _bf16, tag="S")
                            if "sb" not in SKIP:
                                nc.vector.tensor_scalar(S[:], iota_bf[:],
                                                        doft[:, t:t + 1],
                                                        ewt[:, t:t + 1],
                                                        op0=eq, op1=mul)
                            if "mm" not in SKIP:
                                nc.tensor.matmul(cur_ps[:], lhsT=S[:],
                                                 rhs=gbuf[:, s_i, :],
                                                 start=first, stop=last)
                            if last:
                                if w == 0:
                                    nc.scalar.copy(acc_sb[:, b, :], cur_ps[:])
                                else:
                                    epilogue(b, cur_ps)
                                cur_ps = None
                        t0 += nt
                        nt_left -= nt

                with tc.tile_pool(name="p1x", bufs=2) as p1x, \
                     tc.tile_pool(name="p1h", bufs=3) as p1h, \
                     tc.tile_pool(name="p1ps", bufs=3, space="PSUM") as p1ps, \
                     tc.tile_pool(name="pb", bufs=8) as pb, \
                     tc.tile_pool(name="gb", bufs=GBUFS) as gbp, \
                     tc.tile_pool(name="pbps", bufs=4, space="PSUM") as pbps, \
                     tc.tile_pool(name="ep", bufs=2) as ep:
                    p1_window(0)
                    p1_window(1)
                    edge_window(0)
                    edge_window(1)
                    # blocks with no window-1 tiles: epilogue from acc only
                    for b in range(NB):
                        if runs[b][1] is None:
                            epilogue(b, None)

            if reps == 1:
                _phases()
            else:
                with tc.For_i(0, reps, 1):
                    _phases()

    nc.compile()
    return nc


def _core_maps(x, weight, bias, gamma, beta, ew_t, dof_t, idx16):
    """Per-core input maps (shared by kernel() and test.py)."""
    xt = np.zeros((128, SRC_PAD), ml_dtypes.bfloat16)
    xt[:, :N] = np.asarray(x, np.float32).T.astype(ml_dtypes.bfloat16)
    wb = np.asarray(weight, np.float32).astype(ml_dtypes.bfloat16)
    bias = np.asarray(bias, np.float32).reshape(1, D)
    gamma = np.asarray(gamma, np.float32).reshape(1, D)
    beta = np.asarray(beta, np.float32).reshape(1, D)
    maps = []
    for c in range(C):
        maps.append({
            "xt": xt, "w": wb, "bias": bias, "gamma": gamma, "beta": beta,
            "ewt": np.ascontiguousarray(ew_t[c]),
            "doft": np.ascontiguousarray(dof_t[c]),
            "idx16": np.ascontiguousarray(idx16[c]),
        })
    return maps


_CACHE = {}


def _get_compiled(edge_index, edge_weight):
    P, T, ew_t, dof_t, idx16 = _preprocess(edge_index, edge_weight)
    key = P.tobytes()
    if key not in _CACHE:
        tiles, t0w, Tw = _schedule(P)
        _CACHE[key] = _build(P, T, tiles, t0w, Tw)
    return _CACHE[key], ew_t, dof_t, idx16


def kernel(x, edge_index, edge_weight, weight, adaptive_weight, bias,
           ln_gamma, ln_beta):
    nc, ew_t, dof_t, idx16 = _get_compiled(edge_index, edge_weight)
    in_maps = _core_maps(x, weight, bias, ln_gamma, ln_beta, ew_t, dof_t, idx16)
    res = run_bass_kernel_spmd(nc, in_maps, core_ids=list(range(C)))
    out = np.empty((N, D), np.float32)
    for c in range(C):
        out[c * NPC:(c + 1) * NPC] = res.results[c]["out"][:NPC]
    return out


# revision 13
# speedup vs baseline: 1.5600x; 1.5316x over previous
"""AGCNConv (GNN message passing) distributed Bass kernel for 8 TRN2 NeuronCores.

Reference math:
    h   = x @ W
    aew = edge_weight * sigmoid(adaptive_weight)
    deg = segment_sum(aew, row);  dis = where(deg>0, deg^-1/2, 0)
    out = segment_sum(h[col] * (dis[row]*aew*dis[col])[:,None], row)
    out = LeakyReLU(LayerNorm(out + bias))

Key identities exploited:
  * The global factor s = sigmoid(adaptive_weight) cancels in the symmetric
    normalization, so adaptive_weight is unused.
  * norm_e = dis[row]*ew*dis[col] depends only on edge data -> computed on
    the HOST in _preprocess; the device never computes degrees.

v2 design (no collectives): every core receives the FULL transposed node
matrix xT (bf16, replicated input) and redundantly computes h = x @ W for
all 40960 padded source rows, writing bf16 h to its local DRAM in two
20480-row windows (the int16 gather-index limit). Dest nodes are sharded
8 ways (5120 padded rows per core); edges are routed to their dest core
and grouped by (source window, dest block of 128), sorted by source row
within each group (gather locality), padded to 128-edge tiles with a
schedule shared across all cores (SPMD: one program, per-core tables).

Device pipeline per core:
  1. h window w: xT chunks DMA'd in, 4 matmuls per [128,512] PSUM bank
     (lhsT = xT chunk, rhs = W, both bf16), one ACT copy -> bf16 staging,
     one DMA per 512 rows into h[w] DRAM.
  2. Edge pass window w: dma_gather h[col] rows (4 SWDGE queues rotating,
     CH-tile chunks, GBUFS buffers in flight; measured ~2.9 ns/row), build
     S[e,d] = (iota==dest_off[e])*norm[e] per 128-edge tile in one DVE
     tensor_scalar op (bf16), PSUM-accumulate out_blk += S^T @ G per dest
     block. Window-0 partials parked in SBUF (acc_sb); window-1 completion
     triggers the fused epilogue.
  3. Epilogue per block: +acc, +bias, LayerNorm (ACT accum_out for
     mean/var), gamma/beta, LeakyReLU via scalar_tensor_tensor
     max(x, 0.2x), DMA out.
"""

import sys

if "/opt/trn_rl_repo" not in sys.path:
    sys.path.insert(0, "/opt/trn_rl_repo")

import numpy as np
import ml_dtypes

from concourse import bacc, tile, mybir
from concourse.bass_utils import run_bass_kernel_spmd

# ---- problem constants (hardcoded per the harness contract) ----
N = 40000
E = 640000
D = 128
C = 8                # cores
NPC = 5000           # dest nodes per core
NB = 40              # dest blocks of 128 per core
NPCP = NB * 128      # 5120 padded dest rows per core
SRC_PAD = C * NPCP   # 40960 padded source rows
WIN = SRC_PAD // 2   # 20480 rows per gather window (int16 index limit)
LN_EPS = 1e-5
LEAKY_ALPHA = 0.2

# ---- tunables ----
SKIP = frozenset()   # ablation flags: p1, g, sb, mm, epi
CH = 64              # gather chunk size in 128-edge tiles
NQ = 4               # SWDGE queues (ucode max 4)
GBUFS = 4            # gather buffers in flight
XCH = 16             # node tiles per xT chunk DMA (2048 nodes)
PST = 4              # node tiles per P1 PSUM group ([128,512] bank)

_f32 = mybir.dt.float32
_bf16 = mybir.dt.bfloat16
_i16 = mybir.dt.int16


def _preprocess(edge_index, edge_weight):
    """Host: symmetric normalization, edge routing/grouping, shared padded
    schedule, per-core tile-layout tables (ewt/doft/idx16)."""
    row = np.asarray(edge_index[0], dtype=np.int64)
    col = np.asarray(edge_index[1], dtype=np.int64)
    ew = np.asarray(edge_weight, dtype=np.float32)
    deg = np.bincount(row, weights=ew, minlength=N)
    dis = np.where(deg > 0, deg ** -0.5, 0.0)
    ew = (dis[row] * ew * dis[col]).astype(np.float32)

    core = row // NPC
    lid = row - core * NPC
    blk = lid >> 7
    off = (lid & 127).astype(np.float32)
    win = col // WIN                          # source window 0|1
    crel = (col - win * WIN).astype(np.int16)  # 0..20479

    # group key per edge: (core, window, block)
    key = (core * 2 + win) * NB + blk
    counts = np.bincount(key, minlength=C * 2 * NB).reshape(C, 2, NB)
    # shared schedule: tiles per (window, block) = max over cores, rounded up
    P = 128 * np.ceil(counts.max(axis=0) / 128.0).astype(np.int64)  # [2, NB]
    for b in range(NB):
        if P[0, b] + P[1, b] == 0:
            P[1, b] = 128  # at least one (all-dummy) tile per block

    Ppad = int(P.sum())
    T = Ppad // 128
    gstart = np.concatenate([[0], np.cumsum(P.reshape(-1))])[:-1].reshape(2, NB)

    colr_a = np.zeros((C, Ppad), np.int16)
    ew_a = np.zeros((C, Ppad), np.float32)
    dof_a = np.zeros((C, Ppad), np.float32)

    # sort by (group, source row): order within a group is free, ascending
    # source addresses give the gather slightly better DRAM locality
    order = np.lexsort((crel, key))
    key_s = key[order]
    grp_sizes = counts.reshape(-1)
    grp_off = np.concatenate([[0], np.cumsum(grp_sizes)])[:-1]
    within = np.arange(len(key_s)) - grp_off[key_s]
    c_s = key_s // (2 * NB)
    wb_s = key_s % (2 * NB)
    dest = gstart.reshape(-1)[wb_s] + within
    colr_a[c_s, dest] = crel[order]
    ew_a[c_s, dest] = ew[order]
    dof_a[c_s, dest] = off[order]

    # S tiles precomputed on HOST (static edge data): S[e, d] = norm_e if
    # dest_off_e == d else 0.  Layout [C, 128 edge, T*128] bf16 so tile t is
    # the column slice [:, 128t:128(t+1)] — streamed to SBUF and fed to the
    # PE as lhsT directly.  This keeps DVE silent during the edge pass: DVE
    # work would lock GPSIMD out of the shared SBUF port it needs to write
    # SWDGE gather descriptors, starving the gather DMA.
    st = np.zeros((C, 128, T * 128), ml_dtypes.bfloat16)
    cc = c_s
    tt = dest // 128
    ee = (dest % 128).astype(np.int64)
    dd = dof_a[c_s, dest].astype(np.int64)
    st[cc, ee, tt * 128 + dd] = ew[order].astype(ml_dtypes.bfloat16)
    # dma_gather index layout: idx i of a tile at [i%16, i//16], replicated x8
    A = colr_a.reshape(C, T, 8, 16).transpose(0, 1, 3, 2)                      # [C,T,16,8]
    idx16 = np.tile(A, (1, 1, 8, 1)).transpose(0, 2, 1, 3).reshape(C, 128, 8 * T)
    return P, T, st, np.ascontiguousarray(idx16)


def _schedule(P):
    """Static tile schedule shared by all cores."""
    P = np.asarray(P)
    tiles = []  # global tile idx -> (w, b)
    for w in (0, 1):
        for b in range(NB):
            for _ in range(int(P[w, b]) // 128):
                tiles.append((w, b))
    T = len(tiles)
    t0w = [0, int(P[0].sum()) // 128]
    Tw = [t0w[1], T - t0w[1]]
    return tiles, t0w, Tw


def _build(P, T, tiles, t0w, Tw, sim_single_core=False, reps=1):
    del sim_single_core  # v2 has no collectives; kept for test.py compat
    nc = bacc.Bacc("TRN2", target_bir_lowering=False, debug=False,
                   enable_asserts=True, num_devices=1,
                   num_swdge_queues=NQ, dynamic_dma_scratch_size=65536)

    xt_in = nc.dram_tensor("xt", [128, SRC_PAD], _bf16, kind="ExternalInput").ap()
    w_in = nc.dram_tensor("w", [D, D], _bf16, kind="ExternalInput").ap()
    bias_in = nc.dram_tensor("bias", [1, D], _f32, kind="ExternalInput").ap()
    gamma_in = nc.dram_tensor("gamma", [1, D], _f32, kind="ExternalInput").ap()
    beta_in = nc.dram_tensor("beta", [1, D], _f32, kind="ExternalInput").ap()
    st_in = nc.dram_tensor("st", [128, 128 * T], _bf16, kind="ExternalInput").ap()
    idx_in = nc.dram_tensor("idx16", [128, 8 * T], _i16, kind="ExternalInput").ap()
    out_d = nc.dram_tensor("out", [NPCP, D], _f32, kind="ExternalOutput").ap()

    eq = mybir.AluOpType.is_equal
    mul = mybir.AluOpType.mult
    add = mybir.AluOpType.add
    AF = mybir.ActivationFunctionType

    # per-block window runs: (first_tile, last_tile) or None
    runs = [[None, None] for _ in range(NB)]
    for t, (w, b) in enumerate(tiles):
        if runs[b][w] is None:
            runs[b][w] = [t, t]
        else:
            runs[b][w][1] = t

    with tile.TileContext(nc) as tc:
        with (
            tc.tile_pool(name="const", bufs=1) as cp,
            tc.tile_pool(name="resident", bufs=1) as rp,
            tc.tile_pool(name="dram", bufs=1, space="DRAM") as dp,
        ):
            ones_row = cp.tile([1, 128], _f32)
            nc.vector.memset(ones_row[:], 1.0)
            eps_col = cp.tile([128, 1], _f32)
            nc.vector.memset(eps_col[:], float(LN_EPS))
            w_sb = cp.tile([128, 128], _bf16)
            nc.sync.dma_start(w_sb[:], w_in)

            # broadcast bias/gamma/beta rows to all 128 partitions via matmul
            bias_bc = cp.tile([128, 128], _f32)
            gamma_bc = cp.tile([128, 128], _f32)
            beta_bc = cp.tile([128, 128], _f32)
            with tc.tile_pool(name="bc", bufs=1) as bcp, \
                 tc.tile_pool(name="bcps", bufs=1, space="PSUM") as bcps:
                for src_ap, dst in ((bias_in, bias_bc), (gamma_in, gamma_bc),
                                    (beta_in, beta_bc)):
                    r = bcp.tile([1, 128], _f32, tag="bcrow")
                    nc.sync.dma_start(r[:], src_ap)
                    ps = bcps.tile([128, 128], _f32, tag="bcps")
                    nc.tensor.matmul(ps[:], lhsT=ones_row[:], rhs=r[:],
                                     start=True, stop=True)
                    nc.scalar.copy(dst[:], ps[:])

            idx_sb = rp.tile([128, 8 * T], _i16)
            nc.sync.dma_start(idx_sb[:], idx_in)

            acc_sb = rp.tile([128, NB, 128], _f32)  # window-0 partial sums

            h_a = dp.tile([WIN, D], _bf16)
            h_b = dp.tile([WIN, D], _bf16)
            h_d = [h_a, h_b]

            dmaeng = [nc.sync, nc.scalar]  # HWDGE streams for plain DMAs

            def _phases():
                # ---------------- Phase 1: h[w] = bf16(x @ W) ----------------
                def p1_window(w):
                    if "p1" in SKIP:
                        return
                    nchunks = WIN // (XCH * 128)
                    for ci in range(nchunks):
                        base = w * WIN + ci * XCH * 128
                        xc = p1x.tile([128, XCH * 128], _bf16, tag="xc")
                        dmaeng[ci % 2].dma_start(
                            xc[:], xt_in[:, base:base + XCH * 128])
                        for g in range(XCH // PST):
                            ps = p1ps.tile([128, PST * 128], _f32, tag="hps")
                            for j in range(PST):
                                k = (g * PST + j) * 128
                                nc.tensor.matmul(
                                    ps[:, j * 128:(j + 1) * 128],
                                    lhsT=xc[:, k:k + 128], rhs=w_sb[:],
                                    start=True, stop=True)
                            hst = p1h.tile([128, PST, 128], _bf16, tag="hst")
                            nc.scalar.copy(hst[:], ps[:])
                            r0 = ci * XCH * 128 + g * PST * 128
                            dst = h_d[w][r0:r0 + PST * 128, :].rearrange(
                                "(t p) d -> p t d", p=128)
                            dmaeng[(ci + g) % 2].dma_start(dst, hst[:])

                # ---------------- Edge pass: gather + scatter matmuls --------
                def epilogue(b, ps):
                    if "epi" in SKIP:
                        return
                    has0 = runs[b][0] is not None
                    t4 = ep.tile([128, 128], _f32, tag="e_t4")
                    if has0 and ps is not None:
                        nc.vector.tensor_tensor(t4[:], ps[:], acc_sb[:, b, :],
                                                op=add)
                        nc.vector.tensor_tensor(t4[:], t4[:], bias_bc[:], op=add)
                    elif ps is not None:
                        nc.vector.tensor_tensor(t4[:], ps[:], bias_bc[:], op=add)
                    else:
                        nc.vector.tensor_tensor(t4[:], acc_sb[:, b, :],
                                                bias_bc[:], op=add)
                    nsum = ep.tile([128, 1], _f32, tag="e_ns")
                    nc.vector.tensor_reduce(nsum[:], t4[:],
                                            axis=mybir.AxisListType.X,
                                            op=add, negate=True)
                    nmean = ep.tile([128, 1], _f32, tag="e_nm")
                    nc.scalar.mul(nmean[:], nsum[:], 1.0 / 128.0)
                    t5 = ep.tile([128, 128], _f32, tag="e_t5")
                    nc.scalar.activation(t5[:], t4[:], AF.Identity,
                                         bias=nmean[:], scale=1.0)
                    sq = ep.tile([128, 128], _f32, tag="e_sq")
                    vsum = ep.tile([128, 1], _f32, tag="e_vs")
                    nc.scalar.activation(sq[:], t5[:], AF.Square,
                                         accum_out=vsum[:])
                    sd = ep.tile([128, 1], _f32, tag="e_sd")
                    nc.scalar.activation(sd[:], vsum[:], AF.Sqrt,
                                         scale=1.0 / 128.0, bias=eps_col[:])
                    rstd = ep.tile([128, 1], _f32, tag="e_rs")
                    nc.vector.reciprocal(rstd[:], sd[:])
                    t6 = ep.tile([128, 128], _f32, tag="e_t6")
                    nc.vector.scalar_tensor_tensor(t6[:], t5[:], rstd[:],
                                                   gamma_bc[:], op0=mul, op1=mul)
                    nc.vector.tensor_tensor(t6[:], t6[:], beta_bc[:], op=add)
                    osb = ep.tile([128, 128], _f32, tag="e_o")
                    nc.vector.scalar_tensor_tensor(osb[:], t6[:],
                                                   float(LEAKY_ALPHA), t6[:],
                                                   op0=mul,
                                                   op1=mybir.AluOpType.max)
                    nc.sync.dma_start(out_d[b * 128:(b + 1) * 128, :], osb[:])

                gcall = [0]

                def edge_window(w):
                    win_ap = h_d[w][:]
                    nt_left = Tw[w]
                    t0 = t0w[w]
                    cur_ps = None
                    while nt_left > 0:
                        if nt_left > CH:
                            nt = CH
                        elif w == 1 and nt_left > 24:
                            nt = nt_left - 16
                        else:
                            nt = nt_left
                        gbuf = gbp.tile([128, CH, 128], _bf16, tag="g")
                        if "gs" in SKIP:
                            # ablation: same bytes, sequential HWDGE DMA
                            src = h_d[w][:nt * 128, :].rearrange(
                                "(t p) d -> p t d", p=128)
                            dmaeng[gcall[0] % 2].dma_start(gbuf[:, :nt, :], src)
                            gcall[0] += 1
                        elif "g" in SKIP:
                            nc.vector.memset(gbuf[:], 0.0)
                        else:
                            nc.gpsimd.dma_gather(
                                out_ap=gbuf[:, :nt, :], in_ap=win_ap,
                                idxs_ap=idx_sb[:, 8 * t0:8 * (t0 + nt)],
                                num_idxs=128 * nt, num_idxs_reg=128 * nt,
                                elem_size=128, single_packet=False,
                                queue_num=gcall[0] % NQ)
                            gcall[0] += 1
                        st_sb = stp.tile([128, CH * 128], _bf16, tag="st")
                        if "sb" not in SKIP:
                            dmaeng[gcall[0] % 2].dma_start(
                                st_sb[:, :nt * 128],
                                st_in[:, 128 * t0:128 * (t0 + nt)])
                        else:
                            nc.vector.memset(st_sb[:], 0.0)
                        for s_i in range(nt):
                            t = t0 + s_i
                            b = tiles[t][1]
                            first = runs[b][w][0] == t
                            last = runs[b][w][1] == t
                            if first:
                                cur_ps = pbps.tile([128, 128], _f32, tag="blk")
                                if "mm" in SKIP:
                                    nc.vector.memset(cur_ps[:], 0.0)
                            if "mm" not in SKIP:
                                nc.tensor.matmul(
                                    cur_ps[:],
                                    lhsT=st_sb[:, 128 * s_i:128 * (s_i + 1)],
                                    rhs=gbuf[:, s_i, :],
                                    start=first, stop=last)
                            if last:
                                if w == 0:
                                    nc.scalar.copy(acc_sb[:, b, :], cur_ps[:])
                                else:
                                    epilogue(b, cur_ps)
                                cur_ps = None
                        t0 += nt
                        nt_left -= nt

                with tc.tile_pool(name="p1x", bufs=2) as p1x, \
                     tc.tile_pool(name="p1h", bufs=3) as p1h, \
                     tc.tile_pool(name="p1ps", bufs=3, space="PSUM") as p1ps, \
                     tc.tile_pool(name="stp", bufs=2) as stp, \
                     tc.tile_pool(name="gb", bufs=GBUFS) as gbp, \
                     tc.tile_pool(name="pbps", bufs=4, space="PSUM") as pbps, \
                     tc.tile_pool(name="ep", bufs=2) as ep:
                    p1_window(0)
                    p1_window(1)
                    edge_window(0)
                    edge_window(1)
                    # blocks with no window-1 tiles: epilogue from acc only
                    for b in range(NB):
                        if runs[b][1] is None:
                            epilogue(b, None)

            if reps == 1:
                _phases()
            else:
                with tc.For_i(0, reps, 1):
                    _phases()

    nc.compile()
    return nc


def _core_maps(x, weight, bias, gamma, beta, st, idx16):
    """Per-core input maps (shared by kernel() and test.py)."""
    xt = np.zeros((128, SRC_PAD), ml_dtypes.bfloat16)
    xt[:, :N] = np.asarray(x, np.float32).T.astype(ml_dtypes.bfloat16)
    wb = np.asarray(weight, np.float32).astype(ml_dtypes.bfloat16)
    bias = np.asarray(bias, np.float32).reshape(1, D)
    gamma = np.asarray(gamma, np.float32).reshape(1, D)
    beta = np.asarray(beta, np.float32).reshape(1, D)
    maps = []
    for c in range(C):
        maps.append({
            "xt": xt, "w": wb, "bias": bias, "gamma": gamma, "beta": beta,
            "st": np.ascontiguousarray(st[c]),
            "idx16": np.ascontiguousarray(idx16[c]),
        })
    return maps


_CACHE = {}


def _get_compiled(edge_index, edge_weight):
    P, T, st, idx16 = _preprocess(edge_index, edge_weight)
    key = P.tobytes()
    if key not in _CACHE:
        tiles, t0w, Tw = _schedule(P)
        _CACHE[key] = _build(P, T, tiles, t0w, Tw)
    return _CACHE[key], st, idx16


def kernel(x, edge_index, edge_weight, weight, adaptive_weight, bias,
           ln_gamma, ln_beta):
    nc, st, idx16 = _get_compiled(edge_index, edge_weight)
    in_maps = _core_maps(x, weight, bias, ln_gamma, ln_beta, st, idx16)
    res = run_bass_kernel_spmd(nc, in_maps, core_ids=list(range(C)))
    out = np.empty((N, D), np.float32)
    for c in range(C):
        out[c * NPC:(c + 1) * NPC] = res.results[c]["out"][:NPC]
    return out


# revision 17
# speedup vs baseline: 1.5746x; 1.0094x over previous
"""AGCNConv (GNN message passing) distributed Bass kernel for 8 TRN2 NeuronCores.

Reference math:
    h   = x @ W
    aew = edge_weight * sigmoid(adaptive_weight)
    deg = segment_sum(aew, row);  dis = where(deg>0, deg^-1/2, 0)
    out = segment_sum(h[col] * (dis[row]*aew*dis[col])[:,None], row)
    out = LeakyReLU(LayerNorm(out + bias))

Key identities exploited:
  * The global factor s = sigmoid(adaptive_weight) cancels in the symmetric
    normalization, so adaptive_weight is unused.
  * norm_e = dis[row]*ew*dis[col] depends only on edge data -> computed on
    the HOST in _preprocess; the device never computes degrees.

v2 design (no collectives): every core receives the FULL transposed node
matrix xT (bf16, replicated input) and redundantly computes h = x @ W for
all 40960 padded source rows, writing bf16 h to its local DRAM in two
20480-row windows (the int16 gather-index limit). Dest nodes are sharded
8 ways (5120 padded rows per core); edges are routed to their dest core
and grouped by (source window, dest block of 128), sorted by source row
within each group (gather locality), padded to 128-edge tiles with a
schedule shared across all cores (SPMD: one program, per-core tables).

Device pipeline per core:
  1. h window w: xT chunks DMA'd in, 4 matmuls per [128,512] PSUM bank
     (lhsT = xT chunk, rhs = W, both bf16), one ACT copy -> bf16 staging,
     one DMA per 512 rows into h[w] DRAM.
  2. Edge pass window w: dma_gather h[col] rows (4 SWDGE queues rotating,
     CH-tile chunks, GBUFS buffers in flight; measured ~2.9 ns/row), build
     S[e,d] = (iota==dest_off[e])*norm[e] per 128-edge tile in one DVE
     tensor_scalar op (bf16), PSUM-accumulate out_blk += S^T @ G per dest
     block. Window-0 partials parked in SBUF (acc_sb); window-1 completion
     triggers the fused epilogue.
  3. Epilogue per block: +acc, +bias, LayerNorm (ACT accum_out for
     mean/var), gamma/beta, LeakyReLU via scalar_tensor_tensor
     max(x, 0.2x), DMA out.
"""

import sys

if "/opt/trn_rl_repo" not in sys.path:
    sys.path.insert(0, "/opt/trn_rl_repo")

import numpy as np
import ml_dtypes

from concourse import bacc, tile, mybir
from concourse.bass_utils import run_bass_kernel_spmd

# ---- problem constants (hardcoded per the harness contract) ----
N = 40000
E = 640000
D = 128
C = 8                # cores
NPC = 5000           # dest nodes per core
NB = 40              # dest blocks of 128 per core
NPCP = NB * 128      # 5120 padded dest rows per core
SRC_PAD = C * NPCP   # 40960 padded source rows
WIN = SRC_PAD // 2   # 20480 rows per gather window (int16 index limit)
LN_EPS = 1e-5
LEAKY_ALPHA = 0.2

# ---- tunables ----
SKIP = frozenset()   # ablation flags: p1, g, sb, mm, epi
CH = 64              # gather chunk size in 128-edge tiles
NQ = 4               # SWDGE queues (ucode max 4)
GBUFS = 4            # gather buffers in flight
XCH = 16             # node tiles per xT chunk DMA (2048 nodes)
PST = 4              # node tiles per P1 PSUM group ([128,512] bank)

_f32 = mybir.dt.float32
_bf16 = mybir.dt.bfloat16
_i16 = mybir.dt.int16


def _preprocess(edge_index, edge_weight):
    """Host: symmetric normalization, edge routing/grouping, shared padded
    schedule, per-core tile-layout tables (ewt/doft/idx16)."""
    row = np.asarray(edge_index[0], dtype=np.int64)
    col = np.asarray(edge_index[1], dtype=np.int64)
    ew = np.asarray(edge_weight, dtype=np.float32)
    deg = np.bincount(row, weights=ew, minlength=N)
    dis = np.where(deg > 0, deg ** -0.5, 0.0)
    ew = (dis[row] * ew * dis[col]).astype(np.float32)

    core = row // NPC
    lid = row - core * NPC
    blk = lid >> 7
    off = (lid & 127).astype(np.float32)
    win = col // WIN                          # source window 0|1
    crel = (col - win * WIN).astype(np.int16)  # 0..20479

    # group key per edge: (core, window, block)
    key = (core * 2 + win) * NB + blk
    counts = np.bincount(key, minlength=C * 2 * NB).reshape(C, 2, NB)
    # shared schedule: tiles per (window, block) = max over cores, rounded up
    P = 128 * np.ceil(counts.max(axis=0) / 128.0).astype(np.int64)  # [2, NB]
    for b in range(NB):
        if P[0, b] + P[1, b] == 0:
            P[1, b] = 128  # at least one (all-dummy) tile per block

    Ppad = int(P.sum())
    T = Ppad // 128
    gstart = np.concatenate([[0], np.cumsum(P.reshape(-1))])[:-1].reshape(2, NB)

    colr_a = np.zeros((C, Ppad), np.int16)
    ew_a = np.zeros((C, Ppad), np.float32)
    dof_a = np.zeros((C, Ppad), np.float32)

    # sort by (group, source row): order within a group is free, ascending
    # source addresses give the gather slightly better DRAM locality
    order = np.lexsort((crel, key))
    key_s = key[order]
    grp_sizes = counts.reshape(-1)
    grp_off = np.concatenate([[0], np.cumsum(grp_sizes)])[:-1]
    within = np.arange(len(key_s)) - grp_off[key_s]
    c_s = key_s // (2 * NB)
    wb_s = key_s % (2 * NB)
    dest = gstart.reshape(-1)[wb_s] + within
    colr_a[c_s, dest] = crel[order]
    ew_a[c_s, dest] = ew[order]
    dof_a[c_s, dest] = off[order]

    # S tiles precomputed on HOST (static edge data): S[e, d] = norm_e if
    # dest_off_e == d else 0.  Layout [C, 128 edge, T*128] bf16 so tile t is
    # the column slice [:, 128t:128(t+1)] — streamed to SBUF and fed to the
    # PE as lhsT directly.  This keeps DVE silent during the edge pass: DVE
    # work would lock GPSIMD out of the shared SBUF port it needs to write
    # SWDGE gather descriptors, starving the gather DMA.
    st = np.zeros((C, 128, T * 128), ml_dtypes.bfloat16)
    cc = c_s
    tt = dest // 128
    ee = (dest % 128).astype(np.int64)
    dd = dof_a[c_s, dest].astype(np.int64)
    st[cc, ee, tt * 128 + dd] = ew[order].astype(ml_dtypes.bfloat16)
    # dma_gather index layout: idx i of a tile at [i%16, i//16], replicated x8
    A = colr_a.reshape(C, T, 8, 16).transpose(0, 1, 3, 2)                      # [C,T,16,8]
    idx16 = np.tile(A, (1, 1, 8, 1)).transpose(0, 2, 1, 3).reshape(C, 128, 8 * T)
    return P, T, st, np.ascontiguousarray(idx16)


def _schedule(P):
    """Static tile schedule shared by all cores."""
    P = np.asarray(P)
    tiles = []  # global tile idx -> (w, b)
    for w in (0, 1):
        for b in range(NB):
            for _ in range(int(P[w, b]) // 128):
                tiles.append((w, b))
    T = len(tiles)
    t0w = [0, int(P[0].sum()) // 128]
    Tw = [t0w[1], T - t0w[1]]
    return tiles, t0w, Tw


def _build(P, T, tiles, t0w, Tw, sim_single_core=False, reps=1):
    del sim_single_core  # v2 has no collectives; kept for test.py compat
    nc = bacc.Bacc("TRN2", target_bir_lowering=False, debug=False,
                   enable_asserts=True, num_devices=1,
                   num_swdge_queues=NQ, dynamic_dma_scratch_size=65536)

    xt_in = nc.dram_tensor("xt", [128, SRC_PAD], _bf16, kind="ExternalInput").ap()
    w_in = nc.dram_tensor("w", [D, D], _bf16, kind="ExternalInput").ap()
    bias_in = nc.dram_tensor("bias", [1, D], _f32, kind="ExternalInput").ap()
    gamma_in = nc.dram_tensor("gamma", [1, D], _f32, kind="ExternalInput").ap()
    beta_in = nc.dram_tensor("beta", [1, D], _f32, kind="ExternalInput").ap()
    st_in = nc.dram_tensor("st", [128, 128 * T], _bf16, kind="ExternalInput").ap()
    idx_in = nc.dram_tensor("idx16", [128, 8 * T], _i16, kind="ExternalInput").ap()
    out_d = nc.dram_tensor("out", [NPCP, D], _f32, kind="ExternalOutput").ap()

    eq = mybir.AluOpType.is_equal
    mul = mybir.AluOpType.mult
    add = mybir.AluOpType.add
    AF = mybir.ActivationFunctionType

    # per-block window runs: (first_tile, last_tile) or None
    runs = [[None, None] for _ in range(NB)]
    for t, (w, b) in enumerate(tiles):
        if runs[b][w] is None:
            runs[b][w] = [t, t]
        else:
            runs[b][w][1] = t

    with tile.TileContext(nc) as tc:
        with (
            tc.tile_pool(name="const", bufs=1) as cp,
            tc.tile_pool(name="resident", bufs=1) as rp,
            tc.tile_pool(name="dram", bufs=1, space="DRAM") as dp,
        ):
            ones_row = cp.tile([1, 128], _f32)
            nc.vector.memset(ones_row[:], 1.0)
            eps_col = cp.tile([128, 1], _f32)
            nc.vector.memset(eps_col[:], float(LN_EPS))
            w_sb = cp.tile([128, 128], _bf16)
            nc.sync.dma_start(w_sb[:], w_in)

            # broadcast bias/gamma/beta rows to all 128 partitions via matmul
            bias_bc = cp.tile([128, 128], _f32)
            gamma_bc = cp.tile([128, 128], _f32)
            beta_bc = cp.tile([128, 128], _f32)
            with tc.tile_pool(name="bc", bufs=1) as bcp, \
                 tc.tile_pool(name="bcps", bufs=1, space="PSUM") as bcps:
                for src_ap, dst in ((bias_in, bias_bc), (gamma_in, gamma_bc),
                                    (beta_in, beta_bc)):
                    r = bcp.tile([1, 128], _f32, tag="bcrow")
                    nc.sync.dma_start(r[:], src_ap)
                    ps = bcps.tile([128, 128], _f32, tag="bcps")
                    nc.tensor.matmul(ps[:], lhsT=ones_row[:], rhs=r[:],
                                     start=True, stop=True)
                    nc.scalar.copy(dst[:], ps[:])

            idx_sb = rp.tile([128, 8 * T], _i16)
            nc.sync.dma_start(idx_sb[:], idx_in)

            acc_sb = rp.tile([128, NB, 128], _f32)  # window-0 partial sums

            h_a = dp.tile([WIN, D], _bf16)
            h_b = dp.tile([WIN, D], _bf16)
            h_d = [h_a, h_b]

            dmaeng = [nc.sync, nc.scalar]  # HWDGE streams for plain DMAs

            def _phases():
                # ---------------- Phase 1: h[w] = bf16(x @ W) ----------------
                def p1_window(w):
                    if "p1" in SKIP:
                        return
                    nchunks = WIN // (XCH * 128)
                    for ci in range(nchunks):
                        base = w * WIN + ci * XCH * 128
                        xc = p1x.tile([128, XCH * 128], _bf16, tag="xc")
                        dmaeng[ci % 2].dma_start(
                            xc[:], xt_in[:, base:base + XCH * 128])
                        for g in range(XCH // PST):
                            ps = p1ps.tile([128, PST * 128], _f32, tag="hps")
                            for j in range(PST):
                                k = (g * PST + j) * 128
                                nc.tensor.matmul(
                                    ps[:, j * 128:(j + 1) * 128],
                                    lhsT=xc[:, k:k + 128], rhs=w_sb[:],
                                    start=True, stop=True)
                            hst = p1h.tile([128, PST, 128], _bf16, tag="hst")
                            nc.scalar.copy(hst[:], ps[:])
                            r0 = ci * XCH * 128 + g * PST * 128
                            dst = h_d[w][r0:r0 + PST * 128, :].rearrange(
                                "(t p) d -> p t d", p=128)
                            dmaeng[(ci + g) % 2].dma_start(dst, hst[:])

                # ---------------- Edge pass: gather + scatter matmuls --------
                def epilogue(b):
                    # deferred: runs AFTER the last gather so its DVE ops
                    # can't lock GPSIMD out of SWDGE descriptor generation
                    if "epi" in SKIP:
                        return
                    t4 = ep.tile([128, 128], _f32, tag="e_t4")
                    nc.vector.tensor_tensor(t4[:], acc_sb[:, b, :],
                                            bias_bc[:], op=add)
                    nsum = ep.tile([128, 1], _f32, tag="e_ns")
                    nc.vector.tensor_reduce(nsum[:], t4[:],
                                            axis=mybir.AxisListType.X,
                                            op=add, negate=True)
                    nmean = ep.tile([128, 1], _f32, tag="e_nm")
                    nc.scalar.mul(nmean[:], nsum[:], 1.0 / 128.0)
                    t5 = ep.tile([128, 128], _f32, tag="e_t5")
                    nc.scalar.activation(t5[:], t4[:], AF.Identity,
                                         bias=nmean[:], scale=1.0)
                    sq = ep.tile([128, 128], _f32, tag="e_sq")
                    vsum = ep.tile([128, 1], _f32, tag="e_vs")
                    nc.scalar.activation(sq[:], t5[:], AF.Square,
                                         accum_out=vsum[:])
                    sd = ep.tile([128, 1], _f32, tag="e_sd")
                    nc.scalar.activation(sd[:], vsum[:], AF.Sqrt,
                                         scale=1.0 / 128.0, bias=eps_col[:])
                    rstd = ep.tile([128, 1], _f32, tag="e_rs")
                    nc.vector.reciprocal(rstd[:], sd[:])
                    t6 = ep.tile([128, 128], _f32, tag="e_t6")
                    nc.vector.scalar_tensor_tensor(t6[:], t5[:], rstd[:],
                                                   gamma_bc[:], op0=mul, op1=mul)
                    nc.vector.tensor_tensor(t6[:], t6[:], beta_bc[:], op=add)
                    osb = ep.tile([128, 128], _f32, tag="e_o")
                    nc.vector.scalar_tensor_tensor(osb[:], t6[:],
                                                   float(LEAKY_ALPHA), t6[:],
                                                   op0=mul,
                                                   op1=mybir.AluOpType.max)
                    nc.sync.dma_start(out_d[b * 128:(b + 1) * 128, :], osb[:])

                gcall = [0]

                def edge_window(w):
                    win_ap = h_d[w][:]
                    nt_left = Tw[w]
                    t0 = t0w[w]
                    cur_ps = None
                    while nt_left > 0:
                        nt = min(CH, nt_left)
                        gbuf = gbp.tile([128, CH, 128], _bf16, tag="g")
                        if "gs" in SKIP:
                            # ablation: same bytes, sequential HWDGE DMA
                            src = h_d[w][:nt * 128, :].rearrange(
                                "(t p) d -> p t d", p=128)
                            dmaeng[gcall[0] % 2].dma_start(gbuf[:, :nt, :], src)
                            gcall[0] += 1
                        elif "g" in SKIP:
                            nc.vector.memset(gbuf[:], 0.0)
                        else:
                            nc.gpsimd.dma_gather(
                                out_ap=gbuf[:, :nt, :], in_ap=win_ap,
                                idxs_ap=idx_sb[:, 8 * t0:8 * (t0 + nt)],
                                num_idxs=128 * nt, num_idxs_reg=128 * nt,
                                elem_size=128, single_packet=False,
                                queue_num=gcall[0] % NQ)
                            gcall[0] += 1
                        st_sb = stp.tile([128, CH * 128], _bf16, tag="st")
                        if "sb" not in SKIP:
                            dmaeng[gcall[0] % 2].dma_start(
                                st_sb[:, :nt * 128],
                                st_in[:, 128 * t0:128 * (t0 + nt)])
                        else:
                            nc.vector.memset(st_sb[:], 0.0)
                        for s_i in range(nt):
                            t = t0 + s_i
                            b = tiles[t][1]
                            first = runs[b][w][0] == t
                            last = runs[b][w][1] == t
                            preload = w == 1 and runs[b][0] is not None
                            if first:
                                cur_ps = pbps.tile([128, 128], _f32, tag="blk")
                                if preload and "mm" not in SKIP:
                                    # seed the accumulation with the window-0
                                    # partial so no separate add is needed
                                    nc.scalar.copy(cur_ps[:], acc_sb[:, b, :])
                                if "mm" in SKIP:
                                    nc.vector.memset(cur_ps[:], 0.0)
                            if "mm" not in SKIP:
                                nc.tensor.matmul(
                                    cur_ps[:],
                                    lhsT=st_sb[:, 128 * s_i:128 * (s_i + 1)],
                                    rhs=gbuf[:, s_i, :],
                                    start=first and not preload, stop=last)
                            if last:
                                nc.scalar.copy(acc_sb[:, b, :], cur_ps[:])
                                cur_ps = None
                        t0 += nt
                        nt_left -= nt

                with tc.tile_pool(name="p1x", bufs=2) as p1x, \
                     tc.tile_pool(name="p1h", bufs=3) as p1h, \
                     tc.tile_pool(name="p1ps", bufs=3, space="PSUM") as p1ps, \
                     tc.tile_pool(name="stp", bufs=2) as stp, \
                     tc.tile_pool(name="gb", bufs=GBUFS) as gbp, \
                     tc.tile_pool(name="pbps", bufs=4, space="PSUM") as pbps, \
                     tc.tile_pool(name="ep", bufs=2) as ep:
                    p1_window(0)
                    p1_window(1)
                    edge_window(0)
                    edge_window(1)
                    # deferred epilogues: all DVE work after the last gather
                    for b in range(NB):
                        epilogue(b)

            if reps == 1:
                _phases()
            else:
                with tc.For_i(0, reps, 1):
                    _phases()

    nc.compile()
    return nc


def _core_maps(x, weight, bias, gamma, beta, st, idx16):
    """Per-core input maps (shared by kernel() and test.py)."""
    xt = np.zeros((128, SRC_PAD), ml_dtypes.bfloat16)
    xt[:, :N] = np.asarray(x, np.float32).T.astype(ml_dtypes.bfloat16)
    wb = np.asarray(weight, np.float32).astype(ml_dtypes.bfloat16)
    bias = np.asarray(bias, np.float32).reshape(1, D)
    gamma = np.asarray(gamma, np.float32).reshape(1, D)
    beta = np.asarray(beta, np.float32).reshape(1, D)
    maps = []
    for c in range(C):
        maps.append({
            "xt": xt, "w": wb, "bias": bias, "gamma": gamma, "beta": beta,
            "st": np.ascontiguousarray(st[c]),
            "idx16": np.ascontiguousarray(idx16[c]),
        })
    return maps


_CACHE = {}


def _get_compiled(edge_index, edge_weight):
    P, T, st, idx16 = _preprocess(edge_index, edge_weight)
    key = P.tobytes()
    if key not in _CACHE:
        tiles, t0w, Tw = _schedule(P)
        _CACHE[key] = _build(P, T, tiles, t0w, Tw)
    return _CACHE[key], st, idx16


def kernel(x, edge_index, edge_weight, weight, adaptive_weight, bias,
           ln_gamma, ln_beta):
    nc, st, idx16 = _get_compiled(edge_index, edge_weight)
    in_maps = _core_maps(x, weight, bias, ln_gamma, ln_beta, st, idx16)
    res = run_bass_kernel_spmd(nc, in_maps, core_ids=list(range(C)))
    out = np.empty((N, D), np.float32)
    for c in range(C):
        out[c * NPC:(c + 1) * NPC] = res.results[c]["out"][:NPC]
    return out


# revision 25
# speedup vs baseline: 1.7117x; 1.0871x over previous
"""AGCNConv (GNN message passing) distributed Bass kernel for 8 TRN2 NeuronCores.

Reference math:
    h   = x @ W
    aew = edge_weight * sigmoid(adaptive_weight)
    deg = segment_sum(aew, row);  dis = where(deg>0, deg^-1/2, 0)
    out = segment_sum(h[col] * (dis[row]*aew*dis[col])[:,None], row)
    out = LeakyReLU(LayerNorm(out + bias))

Key identities exploited:
  * The global factor s = sigmoid(adaptive_weight) cancels in the symmetric
    normalization, so adaptive_weight is unused.
  * norm_e = dis[row]*ew*dis[col] depends only on edge data -> computed on
    the HOST in _preprocess; the device never computes degrees.

v2 design (no collectives): every core receives the FULL transposed node
matrix xT (bf16, replicated input) and redundantly computes h = x @ W for
all 40960 padded source rows, writing bf16 h to its local DRAM in two
20480-row windows (the int16 gather-index limit). Dest nodes are sharded
8 ways (5120 padded rows per core); edges are routed to their dest core
and grouped by (source window, dest block of 128), sorted by source row
within each group (gather locality), padded to 128-edge tiles with a
schedule shared across all cores (SPMD: one program, per-core tables).

Device pipeline per core:
  1. h window w: xT chunks DMA'd in, 4 matmuls per [128,512] PSUM bank
     (lhsT = xT chunk, rhs = W, both bf16), one ACT copy -> bf16 staging,
     one DMA per 512 rows into h[w] DRAM.
  2. Edge pass window w: dma_gather h[col] rows (4 SWDGE queues rotating,
     CH-tile chunks, GBUFS buffers in flight; measured ~2.9 ns/row), build
     S[e,d] = (iota==dest_off[e])*norm[e] per 128-edge tile in one DVE
     tensor_scalar op (bf16), PSUM-accumulate out_blk += S^T @ G per dest
     block. Window-0 partials parked in SBUF (acc_sb); window-1 completion
     triggers the fused epilogue.
  3. Epilogue per block: +acc, +bias, LayerNorm (ACT accum_out for
     mean/var), gamma/beta, LeakyReLU via scalar_tensor_tensor
     max(x, 0.2x), DMA out.
"""

import sys

if "/opt/trn_rl_repo" not in sys.path:
    sys.path.insert(0, "/opt/trn_rl_repo")

import numpy as np
import ml_dtypes

from concourse import bacc, tile, mybir
from concourse.bass_utils import run_bass_kernel_spmd

# ---- problem constants (hardcoded per the harness contract) ----
N = 40000
E = 640000
D = 128
C = 8                # cores
NPC = 5000           # dest nodes per core
NB = 40              # dest blocks of 128 per core
NPCP = NB * 128      # 5120 padded dest rows per core
SRC_PAD = C * NPCP   # 40960 padded source rows
WIN = SRC_PAD // 2   # 20480 rows per gather window (int16 index limit)
LN_EPS = 1e-5
LEAKY_ALPHA = 0.2

# ---- tunables ----
SKIP = frozenset()   # ablation flags: p1, g, sb, mm, epi
CH = 32              # gather chunk size in 128-edge tiles
NQ = 4               # SWDGE queues (ucode max 4)
GBUFS = 8            # gather buffers in flight (2 per queue)
XCH = 16             # node tiles per xT chunk DMA (2048 nodes)
PST = 4              # node tiles per P1 PSUM group ([128,512] bank)

_f32 = mybir.dt.float32
_bf16 = mybir.dt.bfloat16
_i16 = mybir.dt.int16


def _preprocess(edge_index, edge_weight):
    """Host: symmetric normalization, edge routing/grouping, shared padded
    schedule, per-core tile-layout tables (ewt/doft/idx16)."""
    row = np.asarray(edge_index[0], dtype=np.int64)
    col = np.asarray(edge_index[1], dtype=np.int64)
    ew = np.asarray(edge_weight, dtype=np.float32)
    deg = np.bincount(row, weights=ew, minlength=N)
    dis = np.where(deg > 0, deg ** -0.5, 0.0)
    ew = (dis[row] * ew * dis[col]).astype(np.float32)

    core = row // NPC
    lid = row - core * NPC
    blk = lid >> 7
    off = (lid & 127).astype(np.float32)
    win = col // WIN                          # source window 0|1
    crel = (col - win * WIN).astype(np.int16)  # 0..20479

    # group key per edge: (core, window, block)
    key = (core * 2 + win) * NB + blk
    counts = np.bincount(key, minlength=C * 2 * NB).reshape(C, 2, NB)
    # shared schedule: tiles per (window, block) = max over cores, rounded up
    P = 128 * np.ceil(counts.max(axis=0) / 128.0).astype(np.int64)  # [2, NB]
    for b in range(NB):
        if P[0, b] + P[1, b] == 0:
            P[1, b] = 128  # at least one (all-dummy) tile per block

    Ppad = int(P.sum())
    T = Ppad // 128
    gstart = np.concatenate([[0], np.cumsum(P.reshape(-1))])[:-1].reshape(2, NB)

    colr_a = np.zeros((C, Ppad), np.int16)
    ew_a = np.zeros((C, Ppad), np.float32)
    dof_a = np.zeros((C, Ppad), np.float32)

    # sort by (group, source row): order within a group is free, ascending
    # source addresses give the gather slightly better DRAM locality
    order = np.lexsort((crel, key))
    key_s = key[order]
    grp_sizes = counts.reshape(-1)
    grp_off = np.concatenate([[0], np.cumsum(grp_sizes)])[:-1]
    within = np.arange(len(key_s)) - grp_off[key_s]
    c_s = key_s // (2 * NB)
    wb_s = key_s % (2 * NB)
    dest = gstart.reshape(-1)[wb_s] + within
    colr_a[c_s, dest] = crel[order]
    ew_a[c_s, dest] = ew[order]
    dof_a[c_s, dest] = off[order]

    # S tiles precomputed on HOST (static edge data): S[e, d] = norm_e if
    # dest_off_e == d else 0.  Layout [C, 128 edge, T*128] bf16 so tile t is
    # the column slice [:, 128t:128(t+1)] — streamed to SBUF and fed to the
    # PE as lhsT directly.  This keeps DVE silent during the edge pass: DVE
    # work would lock GPSIMD out of the shared SBUF port it needs to write
    # SWDGE gather descriptors, starving the gather DMA.
    st = np.zeros((C, 128, T * 128), ml_dtypes.bfloat16)
    cc = c_s
    tt = dest // 128
    ee = (dest % 128).astype(np.int64)
    dd = dof_a[c_s, dest].astype(np.int64)
    st[cc, ee, tt * 128 + dd] = ew[order].astype(ml_dtypes.bfloat16)
    # dma_gather index layout: idx i of a tile at [i%16, i//16], replicated x8
    A = colr_a.reshape(C, T, 8, 16).transpose(0, 1, 3, 2)                      # [C,T,16,8]
    idx16 = np.tile(A, (1, 1, 8, 1)).transpose(0, 2, 1, 3).reshape(C, 128, 8 * T)
    return P, T, st, np.ascontiguousarray(idx16)


def _schedule(P):
    """Static tile schedule shared by all cores."""
    P = np.asarray(P)
    tiles = []  # global tile idx -> (w, b)
    for w in (0, 1):
        for b in range(NB):
            for _ in range(int(P[w, b]) // 128):
                tiles.append((w, b))
    T = len(tiles)
    t0w = [0, int(P[0].sum()) // 128]
    Tw = [t0w[1], T - t0w[1]]
    return tiles, t0w, Tw


def _build(P, T, tiles, t0w, Tw, sim_single_core=False, reps=1):
    del sim_single_core  # v2 has no collectives; kept for test.py compat
    nc = bacc.Bacc("TRN2", target_bir_lowering=False, debug=False,
                   enable_asserts=True, num_devices=1,
                   num_swdge_queues=NQ, dynamic_dma_scratch_size=65536)

    xt_in = nc.dram_tensor("xt", [128, SRC_PAD], _bf16, kind="ExternalInput").ap()
    w_in = nc.dram_tensor("w", [D, D], _bf16, kind="ExternalInput").ap()
    bias_in = nc.dram_tensor("bias", [1, D], _f32, kind="ExternalInput").ap()
    gamma_in = nc.dram_tensor("gamma", [1, D], _f32, kind="ExternalInput").ap()
    beta_in = nc.dram_tensor("beta", [1, D], _f32, kind="ExternalInput").ap()
    st_in = nc.dram_tensor("st", [128, 128 * T], _bf16, kind="ExternalInput").ap()
    idx_in = nc.dram_tensor("idx16", [128, 8 * T], _i16, kind="ExternalInput").ap()
    out_d = nc.dram_tensor("out", [NPCP, D], _f32, kind="ExternalOutput").ap()

    eq = mybir.AluOpType.is_equal
    mul = mybir.AluOpType.mult
    add = mybir.AluOpType.add
    AF = mybir.ActivationFunctionType

    # per-block window runs: (first_tile, last_tile) or None
    runs = [[None, None] for _ in range(NB)]
    for t, (w, b) in enumerate(tiles):
        if runs[b][w] is None:
            runs[b][w] = [t, t]
        else:
            runs[b][w][1] = t

    with tile.TileContext(nc) as tc:
        with (
            tc.tile_pool(name="const", bufs=1) as cp,
            tc.tile_pool(name="resident", bufs=1) as rp,
            tc.tile_pool(name="dram", bufs=1, space="DRAM") as dp,
        ):
            ones_row = cp.tile([1, 128], _f32)
            nc.vector.memset(ones_row[:], 1.0)
            eps_col = cp.tile([128, 1], _f32)
            nc.vector.memset(eps_col[:], float(LN_EPS))
            w_sb = cp.tile([128, 128], _bf16)
            nc.sync.dma_start(w_sb[:], w_in)

            # broadcast bias/gamma/beta rows to all 128 partitions via matmul
            bias_bc = cp.tile([128, 128], _f32)
            gamma_bc = cp.tile([128, 128], _f32)
            beta_bc = cp.tile([128, 128], _f32)
            with tc.tile_pool(name="bc", bufs=1) as bcp, \
                 tc.tile_pool(name="bcps", bufs=1, space="PSUM") as bcps:
                for src_ap, dst in ((bias_in, bias_bc), (gamma_in, gamma_bc),
                                    (beta_in, beta_bc)):
                    r = bcp.tile([1, 128], _f32, tag="bcrow")
                    nc.sync.dma_start(r[:], src_ap)
                    ps = bcps.tile([128, 128], _f32, tag="bcps")
                    nc.tensor.matmul(ps[:], lhsT=ones_row[:], rhs=r[:],
                                     start=True, stop=True)
                    nc.scalar.copy(dst[:], ps[:])

            idx_sb = rp.tile([128, 8 * T], _i16)
            nc.sync.dma_start(idx_sb[:], idx_in)

            acc_sb = rp.tile([128, NB, 128], _f32)  # window-0 partial sums

            h_a = dp.tile([WIN, D], _bf16)
            h_b = dp.tile([WIN, D], _bf16)
            h_d = [h_a, h_b]

            dmaeng = [nc.sync, nc.scalar]  # HWDGE streams for plain DMAs

            def _phases():
                # ---------------- Phase 1: h[w] = bf16(x @ W) ----------------
                def p1_window(w):
                    if "p1" in SKIP:
                        # ablation: fill h with arbitrary bytes in one DMA so
                        # gather reads aren't reads-before-any-write
                        hv = h_d[w][:].rearrange("(t p) d -> p t d", p=128)
                        src = xt_in[:, w * WIN:(w + 1) * WIN].rearrange(
                            "p (t d) -> p t d", d=128)
                        nc.sync.dma_start(hv, src)
                        return
                    nchunks = WIN // (XCH * 128)
                    for ci in range(nchunks):
                        base = w * WIN + ci * XCH * 128
                        xc = p1x.tile([128, XCH * 128], _bf16, tag="xc")
                        dmaeng[ci % 2].dma_start(
                            xc[:], xt_in[:, base:base + XCH * 128])
                        for g in range(XCH // PST):
                            ps = p1ps.tile([128, PST * 128], _f32, tag="hps")
                            for j in range(PST):
                                k = (g * PST + j) * 128
                                nc.tensor.matmul(
                                    ps[:, j * 128:(j + 1) * 128],
                                    lhsT=xc[:, k:k + 128], rhs=w_sb[:],
                                    start=True, stop=True)
                            hst = p1h.tile([128, PST, 128], _bf16, tag="hst")
                            nc.scalar.copy(hst[:], ps[:])
                            r0 = ci * XCH * 128 + g * PST * 128
                            dst = h_d[w][r0:r0 + PST * 128, :].rearrange(
                                "(t p) d -> p t d", p=128)
                            dmaeng[(ci + g) % 2].dma_start(dst, hst[:])

                # ---------------- Edge pass: gather + scatter matmuls --------
                def epilogue(b, bias_src):
                    # gated on bias_src (copied only after the final gather):
                    # epilogue DVE ops would otherwise run mid-pass and lock
                    # GPSIMD out of SWDGE gather-descriptor generation
                    if "epi" in SKIP:
                        return
                    t4 = ep.tile([128, 128], _f32, tag="e_t4")
                    nc.vector.tensor_tensor(t4[:], acc_sb[:, b, :],
                                            bias_src[:], op=add)
                    nsum = ep.tile([128, 1], _f32, tag="e_ns")
                    nc.vector.tensor_reduce(nsum[:], t4[:],
                                            axis=mybir.AxisListType.X,
                                            op=add, negate=True)
                    nmean = ep.tile([128, 1], _f32, tag="e_nm")
                    nc.scalar.mul(nmean[:], nsum[:], 1.0 / 128.0)
                    t5 = ep.tile([128, 128], _f32, tag="e_t5")
                    nc.scalar.activation(t5[:], t4[:], AF.Identity,
                                         bias=nmean[:], scale=1.0)
                    sq = ep.tile([128, 128], _f32, tag="e_sq")
                    vsum = ep.tile([128, 1], _f32, tag="e_vs")
                    nc.scalar.activation(sq[:], t5[:], AF.Square,
                                         accum_out=vsum[:])
                    sd = ep.tile([128, 1], _f32, tag="e_sd")
                    nc.scalar.activation(sd[:], vsum[:], AF.Sqrt,
                                         scale=1.0 / 128.0, bias=eps_col[:])
                    rstd = ep.tile([128, 1], _f32, tag="e_rs")
                    nc.vector.reciprocal(rstd[:], sd[:])
                    t6 = ep.tile([128, 128], _f32, tag="e_t6")
                    nc.vector.scalar_tensor_tensor(t6[:], t5[:], rstd[:],
                                                   gamma_bc[:], op0=mul, op1=mul)
                    nc.vector.tensor_tensor(t6[:], t6[:], beta_bc[:], op=add)
                    osb = ep.tile([128, 128], _f32, tag="e_o")
                    nc.vector.scalar_tensor_tensor(osb[:], t6[:],
                                                   float(LEAKY_ALPHA), t6[:],
                                                   op0=mul,
                                                   op1=mybir.AluOpType.max)
                    nc.sync.dma_start(out_d[b * 128:(b + 1) * 128, :], osb[:])

                gcall = [0]

                def edge_window(w):
                    win_ap = h_d[w][:]
                    nt_left = Tw[w]
                    t0 = t0w[w]
                    cur_ps = None
                    while nt_left > 0:
                        nt = min(CH, nt_left)
                        gbuf = gbp.tile([128, CH, 128], _bf16, tag="g")
                        if "gs" in SKIP:
                            # ablation: same bytes, sequential HWDGE DMA
                            src = h_d[w][:nt * 128, :].rearrange(
                                "(t p) d -> p t d", p=128)
                            dmaeng[gcall[0] % 2].dma_start(gbuf[:, :nt, :], src)
                            gcall[0] += 1
                        elif "g" in SKIP:
                            nc.vector.memset(gbuf[:], 0.0)
                        else:
                            nc.gpsimd.dma_gather(
                                out_ap=gbuf[:, :nt, :], in_ap=win_ap,
                                idxs_ap=idx_sb[:, 8 * t0:8 * (t0 + nt)],
                                num_idxs=128 * nt, num_idxs_reg=128 * nt,
                                elem_size=128, single_packet=False,
                                queue_num=gcall[0] % NQ)
                            gcall[0] += 1
                        if "sb" not in SKIP:  # "sb" skip implies "mm" skip
                            st_sb = stp.tile([128, CH * 128], _bf16, tag="st")
                            dmaeng[gcall[0] % 2].dma_start(
                                st_sb[:, :nt * 128],
                                st_in[:, 128 * t0:128 * (t0 + nt)])
                        for s_i in range(nt):
                            t = t0 + s_i
                            b = tiles[t][1]
                            first = runs[b][w][0] == t
                            last = runs[b][w][1] == t
                            preload = w == 1 and runs[b][0] is not None
                            if first:
                                cur_ps = pbps.tile([128, 128], _f32, tag="blk")
                                if preload and "mm" not in SKIP:
                                    # seed the accumulation with the window-0
                                    # partial so no separate add is needed
                                    nc.scalar.copy(cur_ps[:], acc_sb[:, b, :])
                                if "mm" in SKIP:
                                    nc.vector.memset(cur_ps[:], 0.0)
                            if "mm" not in SKIP:
                                nc.tensor.matmul(
                                    cur_ps[:],
                                    lhsT=st_sb[:, 128 * s_i:128 * (s_i + 1)],
                                    rhs=gbuf[:, s_i, :],
                                    start=first and not preload, stop=last)
                            if last:
                                nc.scalar.copy(acc_sb[:, b, :], cur_ps[:])
                                cur_ps = None
                        t0 += nt
                        nt_left -= nt

                with tc.tile_pool(name="p1x", bufs=2) as p1x, \
                     tc.tile_pool(name="p1h", bufs=3) as p1h, \
                     tc.tile_pool(name="p1ps", bufs=3, space="PSUM") as p1ps, \
                     tc.tile_pool(name="stp", bufs=3) as stp, \
                     tc.tile_pool(name="gb", bufs=GBUFS) as gbp, \
                     tc.tile_pool(name="pbps", bufs=4, space="PSUM") as pbps, \
                     tc.tile_pool(name="ep", bufs=2) as ep:
                    # p1(0), edge(0), p1(1), edge(1): keeps PE's in-order
                    # queue from parking edge-A matmuls behind P1-B, which
                    # would stall gather-A once the gather buffers fill
                    p1_window(0)
                    edge_window(0)
                    p1_window(1)
                    edge_window(1)
                    # bias_gated is written after the last edge-pass psum
                    # copy in ACT program order; every epilogue reads it, so
                    # no epilogue DVE op can start before the gathers end
                    bias_gated = ep.tile([128, 128], _f32, tag="e_bg")
                    nc.scalar.copy(bias_gated[:], bias_bc[:])
                    for b in range(NB):
                        epilogue(b, bias_gated)

            if reps == 1:
                _phases()
            else:
                with tc.For_i(0, reps, 1):
                    _phases()

    nc.compile()
    return nc


def _core_maps(x, weight, bias, gamma, beta, st, idx16):
    """Per-core input maps (shared by kernel() and test.py)."""
    xt = np.zeros((128, SRC_PAD), ml_dtypes.bfloat16)
    xt[:, :N] = np.asarray(x, np.float32).T.astype(ml_dtypes.bfloat16)
    wb = np.asarray(weight, np.float32).astype(ml_dtypes.bfloat16)
    bias = np.asarray(bias, np.float32).reshape(1, D)
    gamma = np.asarray(gamma, np.float32).reshape(1, D)
    beta = np.asarray(beta, np.float32).reshape(1, D)
    maps = []
    for c in range(C):
        maps.append({
            "xt": xt, "w": wb, "bias": bias, "gamma": gamma, "beta": beta,
            "st": np.ascontiguousarray(st[c]),
            "idx16": np.ascontiguousarray(idx16[c]),
        })
    return maps


_CACHE = {}


def _get_compiled(edge_index, edge_weight):
    P, T, st, idx16 = _preprocess(edge_index, edge_weight)
    key = P.tobytes()
    if key not in _CACHE:
        tiles, t0w, Tw = _schedule(P)
        _CACHE[key] = _build(P, T, tiles, t0w, Tw)
    return _CACHE[key], st, idx16


def kernel(x, edge_index, edge_weight, weight, adaptive_weight, bias,
           ln_gamma, ln_beta):
    nc, st, idx16 = _get_compiled(edge_index, edge_weight)
    in_maps = _core_maps(x, weight, bias, ln_gamma, ln_beta, st, idx16)
    res = run_bass_kernel_spmd(nc, in_maps, core_ids=list(range(C)))
    out = np.empty((N, D), np.float32)
    for c in range(C):
        out[c * NPC:(c + 1) * NPC] = res.results[c]["out"][:NPC]
    return out
